# revision 1
# baseline (speedup 1.0000x reference)
"""Trainium2 Bass kernel v2: FAVOR (Performer) causal linear attention block.

Per batch element (data-parallel over 8 NeuronCores):
  c = x @ w_inp + b_inp; q,k,v = split(c)
  qf/kf = rfm_softmax(q/k, omega)             (FAVOR random feature maps)
  a     = causal_linear_attention(qf, kf, v)  (prefix outer-products + masked
                                               diagonal blocks)
  out   = a @ w_out + b_out

Design:
  - weights host-cast (bf16 / scaled fp8) and pre-laid-out for [128, *] DMA
  - qk projection runs as fp8e4 DoubleRow matmuls (2 k-planes per pass,
    0.5 cycles/row); weights pre-scaled by 64 to sit in fp8 normal range,
    un-scaled in the PSUM->SBUF activation copy
  - all transposes use a bf16 identity (1 cycle/row on PE)
  - feature maps: exp applied straight from PSUM, per-(l,h) bias folded into
    a post-exp scalar multiply; q-side max skipped (cancels in a/denom)
  - v stored unpadded; attention matmuls use 64-wide lhsT slices with
    partition-offset PSUM outputs
  - off-diagonal attention via per-block prefix sums of kf^T v outer
    products; i-outer pipeline fuses K1/denominator, qf scaling, qf
    transposes, attention, output projection and the out DMA per l-block
"""

import numpy as np
from contextlib import ExitStack

import concourse.bass as bass
import concourse.tile as tile
from concourse import mybir
from concourse import bass_utils
import bass_rust

F32 = mybir.dt.float32
F32R = mybir.dt.float32r
BF16 = mybir.dt.bfloat16
F8 = mybir.dt.float8e4
AF = mybir.ActivationFunctionType
ALU = mybir.AluOpType
DR = mybir.MatmulPerfMode.DoubleRow

B, L, E, H, Dh, F = 8, 512, 768, 12, 64, 64
LT = L // 128      # 4 l-chunks
ET = E // 128      # 6 e-chunks
NH2 = H // 2       # 6 head pairs
EPS = 1e-6
LN8 = 2.0794415416798357   # 0.5 * ln(F)
SCALE_D = float(Dh) ** -0.25
EPSP = EPS * (float(F) ** -0.5)
W8SCALE = 64.0


def _fix_waits(nc, cap=1):
    """Walrus codegen allows a single sync-wait per instruction; hoist excess
    waits onto injected same-engine NoOps placed directly before the offender
    (no reordering, deadlock-free)."""
    n = 0
    for fn in nc.m.functions:
        for bb in fn.blocks:
            insts = bb.instructions
            i = 0
            while i < len(insts):
                inst = insts[i]
                si = inst.sync_info
                if si is not None:
                    ow = list(si.on_wait)
                    if len(ow) > cap:
                        excess, keep = ow[:-cap], ow[-cap:]
                        si.on_wait = keep
                        for w in excess:
                            n += 1
                            nop = bass_rust.InstNoOp(
                                name=f"waitnop_{n}",
                                engine=inst.engine,
                                sync_info=bass_rust.SyncInfo(
                                    on_wait=[w], on_update=[]),
                            )
                            insts.insert(i, nop)
                            i += 1
                i += 1
    return n


class _PhaseCut(Exception):
    pass


def build_nc(fix_waits=True, phases=99, zb=True):
    nc = bass.Bass("TRN2", target_bir_lowering=False, debug=False,
                   num_devices=8)

    x_d = nc.dram_tensor("x", [L, E], F32, kind="ExternalInput").ap()
    wqk_d = nc.dram_tensor("wqk", [128, ET * 1536], BF16,
                           kind="ExternalInput").ap()
    wv_d = nc.dram_tensor("wv", [128, ET * 768], BF16,
                          kind="ExternalInput").ap()
    wo_d = nc.dram_tensor("wo", [128, ET * 768], BF16,
                          kind="ExternalInput").ap()
    ones1_d = nc.dram_tensor("ones1", [1, 128], F32R,
                             kind="ExternalInput").ap()
    wsum_d = nc.dram_tensor("wsum", [128, ET * 24], BF16,
                            kind="ExternalInput").ap()
    if not zb:
        b_vv_d = nc.dram_tensor("b_vv", [128, E], F32,
                                kind="ExternalInput").ap()
        b_orow_d = nc.dram_tensor("b_orow", [1, E], F32R,
                                  kind="ExternalInput").ap()
        bs_d = nc.dram_tensor("bs_rows", [2, 1536 + 24], F32R,
                              kind="ExternalInput").ap()
    idb_d = nc.dram_tensor("idb", [128, 128], BF16, kind="ExternalInput").ap()
    idr_d = nc.dram_tensor("idr", [128, 128], F32R, kind="ExternalInput").ap()
    maskd_d = nc.dram_tensor("mask_diag", [128, 128], BF16,
                             kind="ExternalInput").ap()
    maskf_d = nc.dram_tensor("mask_f", [128, 256], F32,
                             kind="ExternalInput").ap()
    ones_d = nc.dram_tensor("ones128", [128, 128], BF16,
                            kind="ExternalInput").ap()
    out_d = nc.dram_tensor("out", [L, E], F32, kind="ExternalOutput").ap()

    with tile.TileContext(nc) as tc, ExitStack() as ctx:
      try:
        P = ctx.enter_context(tc.tile_pool(name="persist", bufs=1))
        st_p = ctx.enter_context(tc.tile_pool(name="stp", bufs=8))
        sm_p = ctx.enter_context(tc.tile_pool(name="smp", bufs=10))
        dn_p = ctx.enter_context(tc.tile_pool(name="dnp", bufs=2))
        osb_p = ctx.enter_context(tc.tile_pool(name="osb", bufs=2))
        ps = ctx.enter_context(tc.tile_pool(name="ps", bufs=1, space="PSUM"))

        cnt = [0]

        def pst(shape, dtype=F32, tag="big", bufs=5):
            cnt[0] += 1
            return ps.tile(shape, dtype, tag=tag, bufs=bufs,
                           name=f"pst{cnt[0]}")

        def psts(shape, dtype=F32):
            return pst(shape, dtype, tag="small", bufs=3)

        # PSUM is bank-granular: every live tile costs a full 2KB bank.
        # tag "big" x6 + tag "small" x2 = 8 banks.  Small outputs are packed
        # into shared bank tiles (sB+pd, N_j triples, pa columns + pq).

        # Act-table warmup: absorb the 1.3us activation table load at t=0
        warm = P.tile([128, 1], F32, tag="warm", name="warm")
        nc.gpsimd.memset(warm, 0.0)
        nc.scalar.activation(warm, warm, AF.Exp)

        # ---------------- DMAs ----------------
        idb = P.tile([128, 128], BF16, tag="idb", name="idb")
        nc.sync.dma_start(out=idb, in_=idb_d)
        # x: f32 DRAM -> bf16 SBUF cast loads (gpsimd SWDGE), 1 l-chunk/DMA
        xin = [P.tile([128, E], BF16, tag=f"xin{c}", name=f"xin{c}")
               for c in range(LT)]
        for c in range(LT):
            nc.gpsimd.dma_start(out=xin[c],
                                in_=x_d[c * 128:(c + 1) * 128, :])
        wqk = P.tile([128, ET * 1536], BF16, tag="wqk", name="wqk")
        for et in range(ET):
            nc.sync.dma_start(out=wqk[:, et * 1536:(et + 1) * 1536],
                              in_=wqk_d[:, et * 1536:(et + 1) * 1536])
        idr = P.tile([128, 128], F32R, tag="idr", name="idr")
        nc.sync.dma_start(out=idr, in_=idr_d)
        wsum = P.tile([128, ET * 24], BF16, tag="wsum", name="wsum")
        nc.sync.dma_start(out=wsum, in_=wsum_d)
        if not zb:
            bs_rows = P.tile([2, 1536 + 24], F32R, tag="bs_rows",
                             name="bs_rows")
            nc.sync.dma_start(out=bs_rows, in_=bs_d)
        wv = P.tile([128, ET * 768], BF16, tag="wv", name="wv")
        for et in range(ET):
            nc.sync.dma_start(out=wv[:, et * 768:(et + 1) * 768],
                              in_=wv_d[:, et * 768:(et + 1) * 768])
        maskd = P.tile([128, 128], BF16, tag="maskd", name="maskd")
        nc.sync.dma_start(out=maskd, in_=maskd_d)
        maskf2 = P.tile([128, 256], F32, tag="maskf", name="maskf")
        nc.sync.dma_start(out=maskf2, in_=maskf_d)
        ones128 = P.tile([128, 128], BF16, tag="ones", name="ones")
        nc.sync.dma_start(out=ones128, in_=ones_d)
        if not zb:
            b_vv = P.tile([128, E], F32, tag="b_vv", name="b_vv")
            nc.sync.dma_start(out=b_vv, in_=b_vv_d)
        if not zb:
            b_orow = P.tile([1, E], F32R, tag="b_orow", name="b_orow")
            nc.sync.dma_start(out=b_orow, in_=b_orow_d)
        ones1 = P.tile([1, 128], F32R, tag="ones1", name="ones1")
        nc.sync.dma_start(out=ones1, in_=ones1_d)
        wo = P.tile([128, ET * 768], BF16, tag="wo", name="wo")
        for et in range(ET):
            nc.sync.dma_start(out=wo[:, et * 768:(et + 1) * 768],
                              in_=wo_d[:, et * 768:(et + 1) * 768])

        # ---------------- x transposes (dual bf16 + fp8 copies) ------------
        xT_all = P.tile([128, ET * L], BF16, tag="xT_all", name="xT_all")
        xT = [xT_all[:, et * L:(et + 1) * L] for et in range(ET)]
        xTv = xT_all.rearrange("p (et l) -> p et l", l=L)
        rot = [0]

        def spread(dst, src):
            """Copy PSUM->SBUF on a rotating engine (DVE/Act; GPSIMD cannot
            access PSUM)."""
            r = rot[0] % 2
            rot[0] += 1
            if r == 0:
                nc.vector.tensor_copy(dst, src)
            else:
                nc.scalar.copy(dst, src)

        for lt in range(LT):
            pA = pst([128, 512], BF16)
            pB = pst([128, 256], BF16)
            for et in range(ET):
                dst = (pA[:, (et % 4) * 128:(et % 4) * 128 + 128] if et < 4
                       else pB[:, (et - 4) * 128:(et - 4) * 128 + 128])
                nc.tensor.transpose(
                    dst, xin[lt][:, et * 128:(et + 1) * 128], idb)
            spread(xTv[:, 0:4, lt * 128:(lt + 1) * 128],
                   pA.rearrange("p (e l) -> p e l", l=128))
            spread(xTv[:, 4:6, lt * 128:(lt + 1) * 128],
                   pB.rearrange("p (e l) -> p e l", l=128))

        if phases < 1:
            raise _PhaseCut
        # ---------------- fused QKV+omega feature GEMM ----------------
        # s[l, hf] = x @ (W_qk Omega~) accumulated per l-chunk; diag comes
        # from the host-precomputed per-head column sums (wsum).
        def feature_lt(qk, dst_t, fac_q, lt):
            # s[l, hf] = x @ Ws (+ b_s); kf = fac_k*exp(s) + EPSP with
            # fac_k = F^-.5 exp(-diag - m_k); qf left raw (factor folded
            # into the denominator pass).  m_q skipped: cancels in a/denom.
            if True:
                sA = pst([128, 512])
                sB = psts([128, 256])
                pd = psts([128, 12])
                xsl = lambda et: xT[et][:, lt * 128:(lt + 1) * 128]
                c0 = qk * 768
                if not zb:
                    nc.tensor.matmul(sA, ones1, bs_rows[qk:qk + 1, 0:512],
                                     start=True, stop=False,
                                     skip_group_check=True)
                    nc.tensor.matmul(sB, ones1, bs_rows[qk:qk + 1, 512:768],
                                     start=True, stop=False,
                                     skip_group_check=True)
                    nc.tensor.matmul(pd, ones1,
                                     bs_rows[qk:qk + 1, 1536:1548],
                                     start=True, stop=False,
                                     skip_group_check=True)
                for et in range(ET):
                    st0 = (et == 0) and zb
                    sp = (et == ET - 1)
                    nc.tensor.matmul(
                        sA, xsl(et),
                        wqk[:, et * 1536 + c0:et * 1536 + c0 + 512],
                        start=st0, stop=sp, skip_group_check=True)
                    nc.tensor.matmul(
                        sB, xsl(et),
                        wqk[:, et * 1536 + c0 + 512:et * 1536 + c0 + 768],
                        start=st0, stop=sp, skip_group_check=True)
                    nc.tensor.matmul(
                        pd, xsl(et),
                        wsum[:, et * 24 + qk * 12:et * 24 + (qk + 1) * 12],
                        start=st0, stop=sp, skip_group_check=True)
                bp = sm_p.tile([128, 12], F32, tag="bp", name="bp")
                nc.vector.tensor_scalar(bp, pd, -0.5, -LN8,
                                        op0=ALU.mult, op1=ALU.add)
                if qk == 1:
                    mk1 = sm_p.tile([128, 2], F32, tag="mk1", name="mk1")
                    nc.vector.reduce_max(mk1[:, 0:1], sA,
                                         axis=mybir.AxisListType.X)
                    nc.vector.reduce_max(mk1[:, 1:2], sB,
                                         axis=mybir.AxisListType.X)
                    mk = sm_p.tile([128, 1], F32, tag="mk", name="mk")
                    nc.vector.reduce_max(mk, mk1, axis=mybir.AxisListType.X)
                    nc.vector.tensor_sub(bp, bp, mk.to_broadcast((128, 12)))
                fac = sm_p.tile([128, 12], F32, tag="fac", name="fac")
                nc.scalar.activation(fac, bp, AF.Exp)
                dst = dst_t[lt]
                nc.scalar.activation(dst[:, 0:512], sA, AF.Exp)
                nc.scalar.activation(dst[:, 512:768], sB, AF.Exp)
                if qk == 1:
                    for h in range(H):
                        sl = dst[:, h * 64:(h + 1) * 64]
                        nc.gpsimd.tensor_scalar(
                            sl, sl, fac[:, h:h + 1], EPSP,
                            op0=ALU.mult, op1=ALU.add)
                else:
                    fac_q[lt] = fac

        kf = [P.tile([128, H * F], BF16, tag=f"kf{lt}", name=f"kf{lt}")
              for lt in range(LT)]
        qf = [P.tile([128, H * F], F32R, tag=f"qf{lt}", name=f"qf{lt}")
              for lt in range(LT)]
        qf_b = [P.tile([128, H * F], BF16, tag=f"qfb{lt}", name=f"qfb{lt}")
                for lt in range(LT)]
        fac_q = [None] * LT

        if phases < 2:
            raise _PhaseCut
        for lt in range(LT):
            feature_lt(1, kf, None, lt)
        if phases < 3:
            raise _PhaseCut
        # kf -> [f, l] head-pair transposes via the DMA XBAR (2-byte SBUF
        # transpose): frees PE and the evacuation copies entirely
        kfT_all = P.tile([128, NH2 * L], BF16, tag="kfT_all", name="kfT_all")
        kfT = [kfT_all[:, t * L:(t + 1) * L] for t in range(NH2)]
        for lt in range(LT):
            for t in range(NH2):
                nc.sync.dma_start(
                    out=kfT[t][:, lt * 128:(lt + 1) * 128],
                    in_=kf[lt][:, t * 128:(t + 1) * 128], transpose=True)

        if phases < 4:
            raise _PhaseCut
        if phases < 6:
            raise _PhaseCut
        # ------- denominator pipeline: K1, den, qf scale, qfT -------
        qfT_all = P.tile([128, NH2 * L], BF16, tag="qfT_all", name="qfT_all")
        qfT = [qfT_all[:, t * L:(t + 1) * L] for t in range(NH2)]
        qfTv = qfT_all.rearrange("p (t l) -> p t l", l=L)
        def qft_block(i):
            for t in range(NH2):
                eng = nc.sync if t % 2 == 0 else nc.scalar
                eng.dma_start(
                    out=qfT[t][:, i * 128:(i + 1) * 128],
                    in_=qf_b[i][:, t * 128:(t + 1) * 128], transpose=True)

        for i in range(LT):
            feature_lt(0, qf, fac_q, i)
            ka = pst([128, 384])
            kb = pst([128, 384])
            for j in range(i + 1):
                m = ones128 if j < i else maskd
                nc.tensor.matmul(ka, m, kf[j][:, 0:384],
                                 start=(j == 0), stop=(j == i))
                nc.tensor.matmul(kb, m, kf[j][:, 384:768],
                                 start=(j == 0), stop=(j == i))
            dn = dn_p.tile([128, H * F], F32, tag="dn", name="dn")
            nc.vector.tensor_mul(dn[:, 0:384], qf[i][:, 0:384], ka)
            nc.vector.tensor_mul(dn[:, 384:768], qf[i][:, 384:768], kb)
            den = sm_p.tile([128, 12], F32, tag="den", name="den")
            nc.vector.reduce_sum(den, dn.rearrange("p (h f) -> p h f", f=64),
                                 axis=mybir.AxisListType.X)
            # den_true = fac_q * den ; rq = fac_q / (den_true + EPS)
            nc.vector.tensor_mul(den, den, fac_q[i])
            nc.vector.tensor_scalar_add(den, den, EPS)
            rq = sm_p.tile([128, 12], F32, tag="rq", name="rq")
            with nc.allow_low_precision(reason="recip of O(1) denom"):
                nc.vector.reciprocal(rq, den)
            nc.vector.tensor_mul(rq, rq, fac_q[i])
            for h in range(H):
                nc.gpsimd.tensor_scalar_mul(
                    qf_b[i][:, h * 64:(h + 1) * 64],
                    qf[i][:, h * 64:(h + 1) * 64], rq[:, h:h + 1])
            qft_block(i)


        # ---------------- QKV: v natural [l, (h d)] ----------------
        v_p = [P.tile([128, E], BF16, tag=f"vp{lt}", name=f"vp{lt}")
               for lt in range(LT)]
        NP = [[None] * 3 for t in range(NH2)]
        for t in range(NH2):
            for i in (1, 2):
                NP[t][i] = P.tile([128, F], BF16, tag=f"NP{t}_{i}",
                                  name=f"NP{t}_{i}")
        pns = {}
        pnbs = {}
        for lt in range(LT):
            for nh in range(2):
                pv = pst([128, 384])
                for et in range(ET):
                    nc.tensor.matmul(
                        pv, xT[et][:, lt * 128:(lt + 1) * 128],
                        wv[:, et * 768 + nh * 384:et * 768 + (nh + 1) * 384],
                        start=(et == 0), stop=(et == ET - 1))
                dst = v_p[lt][:, nh * 384:(nh + 1) * 384]
                if zb:
                    # late blocks evacuate on Act: keeps DVE clear for the
                    # first attention score mask-muls
                    if lt < 2:
                        nc.vector.tensor_copy(dst, pv)
                    else:
                        nc.scalar.copy(dst, pv)
                else:
                    nc.vector.tensor_add(dst, pv,
                                         b_vv[:, nh * 384:(nh + 1) * 384])
            # N_j outer products for this l-chunk (j < LT-1 used by prefixes)
            if lt < LT - 1:
                for t in range(NH2):
                    if lt == 0 and t % 2 == 0:
                        pns[t // 2] = psts([128, 2 * 3 * F])
                    pn = pns[t // 2][:, (t % 2) * 3 * F:(t % 2 + 1) * 3 * F]
                    for hh in range(2):
                        h = 2 * t + hh
                        nc.tensor.matmul(
                            pn[hh * 64:hh * 64 + 64, lt * F:(lt + 1) * F],
                            kf[lt][:, h * 64:(h + 1) * 64],
                            v_p[lt][:, h * 64:(h + 1) * 64],
                            start=True, stop=True)
        for tp in range(NH2 // 2):
            pnb = P.tile([128, 6 * F], BF16, tag=f"pnb{tp}", name=f"pnb{tp}")
            pnbs[tp] = pnb
            nc.scalar.copy(pnb, pns[tp])
        for t in range(NH2):
            pnb = pnbs[t // 2]
            base = (t % 2) * 3 * F
            NP[t][0] = pnb[:, base:base + F]
            nc.gpsimd.tensor_add(NP[t][1], NP[t][0],
                                 pnb[:, base + F:base + 2 * F])
            nc.gpsimd.tensor_add(NP[t][2], NP[t][1],
                                 pnb[:, base + 2 * F:base + 3 * F])

        if phases < 5:
            raise _PhaseCut

        if phases < 7:
            raise _PhaseCut
        # ------- attention (diag masked + prefix) fused with outproj -------
        aTbig = P.tile([128, NH2 * L], BF16, tag="aTbig", name="aTbig")
        aT_all = [aTbig[:, t * L:(t + 1) * L] for t in range(NH2)]
        aTv = aTbig.rearrange("p (t l) -> p t l", l=L)
        for i in range(LT):
            paqA = pst([128, 512])
            paqB = pst([128, 256])
            po = [pst([128, 384]) for _ in range(2)]
            if not zb:
                for nh in range(2):
                    nc.tensor.matmul(
                        po[nh], ones1, b_orow[0:1, nh * 384:(nh + 1) * 384],
                        start=True, stop=False, skip_group_check=True)
            for t in range(NH2):
                pa = (paqA[:, (t % 4) * 128:(t % 4) * 128 + 128] if t < 4
                      else paqB[:, (t - 4) * 128:(t - 4) * 128 + 128])
                sts = []
                for hh in range(2):
                    pq = psts([128, 128])
                    nc.tensor.matmul(
                        pq,
                        kfT[t][hh * 64:hh * 64 + 64, i * 128:(i + 1) * 128],
                        qfT[t][hh * 64:hh * 64 + 64, i * 128:(i + 1) * 128],
                        start=True, stop=True)
                    st = st_p.tile([128, 128], BF16, tag="st", name="st")
                    nc.vector.tensor_mul(st, pq, maskf2[:, 0:128])
                    sts.append(st)
                for hh in range(2):
                    h = 2 * t + hh
                    dst = pa[hh * 64:hh * 64 + 64, :]
                    if i > 0:
                        nc.tensor.matmul(
                            dst, NP[t][i - 1][hh * 64:hh * 64 + 64, :],
                            qfT[t][hh * 64:hh * 64 + 64,
                                   i * 128:(i + 1) * 128],
                            start=True, stop=False, skip_group_check=True)
                    nc.tensor.matmul(
                        dst, v_p[i][:, h * 64:(h + 1) * 64], sts[hh],
                        start=(i == 0), stop=True, skip_group_check=True)
                if t % 2 == 1:
                    if t < 4:
                        nc.scalar.copy(
                            aTv[:, t - 1:t + 1, i * 128:(i + 1) * 128],
                            paqA.rearrange("p (t l) -> p t l", l=128)
                            [:, t - 1:t + 1, :])
                    else:
                        nc.scalar.copy(
                            aTv[:, 4:6, i * 128:(i + 1) * 128],
                            paqB.rearrange("p (t l) -> p t l", l=128))
                    for tt in (t - 1, t):
                        for nh in range(2):
                            nc.tensor.matmul(
                                po[nh], aT_all[tt][:, i * 128:(i + 1) * 128],
                                wo[:, tt * 768 + nh * 384:
                                   tt * 768 + (nh + 1) * 384],
                                start=(zb and tt == 0),
                                stop=(tt == NH2 - 1),
                                skip_group_check=True)
            osb = osb_p.tile([128, E], F32, tag="osb", name="osb")
            for nh in range(2):
                if nh == 0 and i == LT - 1:
                    # final block: DVE/Act split minimizes the kernel tail
                    nc.vector.tensor_copy(osb[:, 0:384], po[0])
                else:
                    # mid blocks: keep DVE free for the next block's score
                    # mask-muls; store latency is hidden by the next block
                    nc.scalar.copy(osb[:, nh * 384:(nh + 1) * 384], po[nh])
                eng = nc.sync if nh == 0 else nc.scalar
                eng.dma_start(
                    out=out_d[i * 128:(i + 1) * 128, nh * 384:(nh + 1) * 384],
                    in_=osb[:, nh * 384:(nh + 1) * 384])
      except _PhaseCut:
        pass

    if fix_waits:
        _fix_waits(nc)
    return nc


_CACHE = {}


def _host_consts():
    import ml_dtypes
    bf = ml_dtypes.bfloat16
    return {
        "idb": np.eye(128, dtype=np.float32).astype(bf),
        "idr": np.eye(128, dtype=np.float32),
        "mask_diag": np.triu(np.ones((128, 128), dtype=np.float32)).astype(bf),
        "mask_f": np.tile(np.triu(np.ones((128, 128), dtype=np.float32)),
                          (1, 2)),
        "ones128": np.ones((128, 128), dtype=bf),
    }


def _in_maps(x, w_inp, b_inp, w_out, b_out, omega):
    import ml_dtypes
    bf = ml_dtypes.bfloat16
    f = lambda a: np.ascontiguousarray(np.asarray(a), dtype=np.float32)
    x, w_inp, b_inp = f(x), f(w_inp), f(b_inp)
    w_out, b_out, omega = f(w_out), f(b_out), f(omega)
    w = w_inp[0]  # [E, 3E]
    omt = (omega.T * SCALE_D).astype(np.float64)   # [d, f]
    # fold omega into the q/k projections: Ws[:, (qk,h,f)] per head
    ws = np.empty((E, 1536), np.float64)
    wqk_full = w[:, 0:1536].astype(np.float64)
    for qk in range(2):
        for h in range(H):
            c = qk * 768 + h * 64
            ws[:, c:c + 64] = wqk_full[:, c:c + 64] @ omt
    wsum_full = ws.reshape(E, 24, 64).sum(axis=2)       # [E, (qk h)]
    wqk = np.ascontiguousarray(
        ws.astype(np.float32).reshape(E, 1536)
        .reshape(ET, 128, 1536).transpose(1, 0, 2)
        .reshape(128, ET * 1536)).astype(bf)
    wsum = np.ascontiguousarray(
        wsum_full.astype(np.float32).reshape(ET, 128, 24).transpose(1, 0, 2)
        .reshape(128, ET * 24)).astype(bf)
    wv = np.ascontiguousarray(
        w[:, 1536:2304].reshape(ET, 128, 768).transpose(1, 0, 2)
        .reshape(128, ET * 768)).astype(bf)
    wo = np.ascontiguousarray(
        w_out[0].reshape(ET, 128, 768).transpose(1, 0, 2)
        .reshape(128, ET * 768)).astype(bf)
    zb = bool(np.all(b_inp == 0.0) and np.all(b_out == 0.0))
    consts = _host_consts()
    maps = []
    for c in range(B):
        m = {"x": x[c], "wqk": wqk, "wv": wv, "wo": wo, "wsum": wsum,
             "ones1": np.ones((1, 128), np.float32)}
        if not zb:
            bs = np.zeros((2, 1536 + 24), np.float32)
            for qk in range(2):
                bq = b_inp[qk * 768:(qk + 1) * 768].astype(np.float64)
                bsh = np.empty((768,), np.float64)
                for h in range(H):
                    bsh[h * 64:(h + 1) * 64] = bq[h * 64:(h + 1) * 64] @ omt
                bs[qk, 0:768] = bsh.astype(np.float32)
                bs[qk, 1536:1548] = (
                    bsh.reshape(12, 64).sum(axis=1).astype(np.float32))
            m["bs_rows"] = bs
            m["b_vv"] = np.ascontiguousarray(
                np.broadcast_to(b_inp[1536:2304], (128, E)))
            m["b_orow"] = np.ascontiguousarray(b_out).reshape(1, E)
        m.update(consts)
        maps.append(m)
    return maps


def kernel(x, w_inp, b_inp, w_out, b_out, omega):
    maps = _in_maps(x, w_inp, b_inp, w_out, b_out, omega)
    zb = "b_vv" not in maps[0]
    key = f"nc{int(zb)}"
    if key not in _CACHE:
        _CACHE[key] = build_nc(zb=zb)
    nc = _CACHE[key]
    res = bass_utils.run_bass_kernel_spmd(nc, maps, core_ids=list(range(B)))
    return np.stack([res.results[c]["out"] for c in range(B)])



# revision 3
# speedup vs baseline: 1.0379x; 1.0379x over previous
"""Trainium2 Bass kernel v3: FAVOR (Performer) causal linear attention block.

Per batch element (data-parallel over 8 NeuronCores):
  c = x @ w_inp + b_inp; q,k,v = split(c)
  qf/kf = rfm_softmax(q/k, omega)             (FAVOR random feature maps)
  a     = causal_linear_attention(qf, kf, v)  (prefix outer-products + masked
                                               diagonal blocks)
  out   = a @ w_out + b_out

v3 design notes:
  - x transposed on host; QKV-feature and V GEMMs run as fp8e4 DoubleRow
    matmuls (2 k-planes per instruction, 0.5 cyc/row); weights pre-scaled
    by 64 into fp8 normal range, un-scaled via exp(s/64) activation scale
    (features) and wo/64 host fold (v path: v'=64v carried through).
  - q-side normalizer exp(-diag-m)/sqrt(F) cancels in a/denom: qf = exp(s_q).
  - k-side max taken as r = rowmax(exp(s_k)) on the bf16 feature tile;
    per-head factor applied as one broadcast DVE multiply.
  - K1 (cumulative kf sums) accumulated in a persistent PSUM pair via
    triu/strict-tril masks: 2 matmuls per block after the first.
  - attention: per-block diag scores (masked on DVE/Pool) + prefix NP
    outer-product matmuls; aT feeds output projection directly as lhsT.
"""

import numpy as np
from contextlib import ExitStack

import concourse.bass as bass
import concourse.tile as tile
from concourse import mybir
from concourse import bass_utils
import bass_rust

F32 = mybir.dt.float32
F32R = mybir.dt.float32r
BF16 = mybir.dt.bfloat16
F8 = mybir.dt.float8e4
AF = mybir.ActivationFunctionType
ALU = mybir.AluOpType
DR = mybir.MatmulPerfMode.DoubleRow

B, L, E, H, Dh, F = 8, 512, 768, 12, 64, 64
LT = L // 128       # 4 l-chunks
ET = E // 128       # 6 e-chunks
PR = ET // 2        # 3 e-pair chunks (DoubleRow planes)
NH2 = H // 2        # 6 head pairs
EPS = 1e-6
W8SCALE = 64.0
IS = 1.0 / W8SCALE


def _fix_waits(nc, cap=1):
    """Walrus codegen allows a single sync-wait per instruction; hoist excess
    waits onto injected same-engine NoOps placed directly before the offender
    (no reordering, deadlock-free)."""
    n = 0
    for fn in nc.m.functions:
        for bb in fn.blocks:
            insts = bb.instructions
            i = 0
            while i < len(insts):
                inst = insts[i]
                si = inst.sync_info
                if si is not None:
                    ow = list(si.on_wait)
                    if len(ow) > cap:
                        excess, keep = ow[:-cap], ow[-cap:]
                        si.on_wait = keep
                        for w in excess:
                            n += 1
                            nop = bass_rust.InstNoOp(
                                name=f"waitnop_{n}",
                                engine=inst.engine,
                                sync_info=bass_rust.SyncInfo(
                                    on_wait=[w], on_update=[]),
                            )
                            insts.insert(i, nop)
                            i += 1
                i += 1
    return n


def build_nc(fix_waits=True, zb=True):
    nc = bass.Bass("TRN2", target_bir_lowering=False, debug=False,
                   num_devices=8)

    x8_d = nc.dram_tensor("x8", [128, PR * 2 * L], F8,
                          kind="ExternalInput").ap()
    wqk8_d = nc.dram_tensor("wqk8", [128, PR * 2 * 1536], F8,
                            kind="ExternalInput").ap()
    wsum8_d = nc.dram_tensor("wsum8", [128, PR * 2 * 16], F8,
                             kind="ExternalInput").ap()
    wv8_d = nc.dram_tensor("wv8", [128, PR * 2 * 768], F8,
                           kind="ExternalInput").ap()
    wo_d = nc.dram_tensor("wo", [128, ET * 768], BF16,
                          kind="ExternalInput").ap()
    maskd_d = nc.dram_tensor("mask_diag", [128, 128], BF16,
                             kind="ExternalInput").ap()
    maskl_d = nc.dram_tensor("mask_low", [128, 128], BF16,
                             kind="ExternalInput").ap()
    maskf_d = nc.dram_tensor("mask_f", [128, 256], BF16,
                             kind="ExternalInput").ap()
    if not zb:
        ones1_d = nc.dram_tensor("ones1", [1, 128], F32R,
                                 kind="ExternalInput").ap()
        bs_d = nc.dram_tensor("bs_rows", [2, 1536 + 16], F32R,
                              kind="ExternalInput").ap()
        b_vv_d = nc.dram_tensor("b_vv", [128, E], F32,
                                kind="ExternalInput").ap()
        b_orow_d = nc.dram_tensor("b_orow", [1, E], F32R,
                                  kind="ExternalInput").ap()
    out_d = nc.dram_tensor("out", [L, E], F32, kind="ExternalOutput").ap()

    with tile.TileContext(nc) as tc, ExitStack() as ctx:
        P = ctx.enter_context(tc.tile_pool(name="persist", bufs=1))
        st_p = ctx.enter_context(tc.tile_pool(name="stp", bufs=6))
        sm_p = ctx.enter_context(tc.tile_pool(name="smp", bufs=8))
        dn_p = ctx.enter_context(tc.tile_pool(name="dnp", bufs=2))
        osb_p = ctx.enter_context(tc.tile_pool(name="osb", bufs=2))
        ps = ctx.enter_context(tc.tile_pool(name="ps", bufs=1, space="PSUM"))

        cnt = [0]

        def pst(shape, dtype=F32, tag="big", bufs=4):
            cnt[0] += 1
            return ps.tile(shape, dtype, tag=tag, bufs=bufs,
                           name=f"pst{cnt[0]}")

        def psts(shape, dtype=F32):
            return pst(shape, dtype, tag="small", bufs=2)

        # PSUM budget: tag big x4 + small x2 + acc x2 = 8 banks.

        # Act-table warmup: absorb the 1.3us activation table load at t=0
        warm = P.tile([128, 1], F32, tag="warm", name="warm")
        nc.gpsimd.memset(warm, 0.0)
        nc.scalar.activation(warm, warm, AF.Exp)

        # ---------------- DMAs ----------------
        # SP queue: the critical-path fp8 operands, chunked by e-pair so the
        # first QKV matmuls start as early as possible.
        x8 = P.tile([128, PR * 2 * L], F8, tag="x8", name="x8")
        x8v = x8.rearrange("p (pr two l) -> p pr two l", two=2, l=L)
        wqk8 = P.tile([128, PR * 2 * 1536], F8, tag="wqk8", name="wqk8")
        wqk8v = wqk8.rearrange("p (pr two c) -> p pr two c", two=2, c=1536)
        for p in range(PR):
            nc.sync.dma_start(out=x8[:, p * 1024:(p + 1) * 1024],
                              in_=x8_d[:, p * 1024:(p + 1) * 1024])
            nc.sync.dma_start(out=wqk8[:, p * 3072:(p + 1) * 3072],
                              in_=wqk8_d[:, p * 3072:(p + 1) * 3072])
        wsum8 = P.tile([128, PR * 2 * 16], F8, tag="wsum8", name="wsum8")
        wsum8v = wsum8.rearrange("p (pr two c) -> p pr two c", two=2, c=16)
        nc.sync.dma_start(out=wsum8, in_=wsum8_d)
        if not zb:
            ones1 = P.tile([1, 128], F32R, tag="ones1", name="ones1")
            nc.sync.dma_start(out=ones1, in_=ones1_d)
            bs_rows = P.tile([2, 1536 + 16], F32R, tag="bs_rows",
                             name="bs_rows")
            nc.sync.dma_start(out=bs_rows, in_=bs_d)

        # Pool (SWDGE) queue: masks + later-phase weights.
        maskd = P.tile([128, 128], BF16, tag="maskd", name="maskd")
        nc.gpsimd.dma_start(out=maskd, in_=maskd_d)
        maskl = P.tile([128, 128], BF16, tag="maskl", name="maskl")
        nc.gpsimd.dma_start(out=maskl, in_=maskl_d)
        maskf = P.tile([128, 256], BF16, tag="maskf", name="maskf")
        nc.gpsimd.dma_start(out=maskf, in_=maskf_d)
        wv8 = P.tile([128, PR * 2 * 768], F8, tag="wv8", name="wv8")
        wv8v = wv8.rearrange("p (pr two c) -> p pr two c", two=2, c=768)
        for p in range(PR):
            nc.gpsimd.dma_start(out=wv8[:, p * 1536:(p + 1) * 1536],
                                in_=wv8_d[:, p * 1536:(p + 1) * 1536])
        wo = P.tile([128, ET * 768], BF16, tag="wo", name="wo")
        for et in range(ET):
            nc.gpsimd.dma_start(out=wo[:, et * 768:(et + 1) * 768],
                                in_=wo_d[:, et * 768:(et + 1) * 768])
        if not zb:
            b_vv = P.tile([128, E], F32, tag="b_vv", name="b_vv")
            nc.gpsimd.dma_start(out=b_vv, in_=b_vv_d)
            b_orow = P.tile([1, E], F32R, tag="b_orow", name="b_orow")
            nc.gpsimd.dma_start(out=b_orow, in_=b_orow_d)

        # ---------------- persistent SBUF tiles ----------------
        kf = [P.tile([128, H * F], BF16, tag=f"kf{lt}", name=f"kf{lt}")
              for lt in range(LT)]
        qf = [P.tile([128, H * F], BF16, tag=f"qf{lt}", name=f"qf{lt}")
              for lt in range(LT)]
        qf_b = [P.tile([128, H * F], BF16, tag=f"qfb{lt}", name=f"qfb{lt}")
                for lt in range(LT)]
        v_p = [P.tile([128, E], BF16, tag=f"vp{lt}", name=f"vp{lt}")
               for lt in range(LT)]
        kfT_all = P.tile([128, NH2 * L], BF16, tag="kfT", name="kfT")
        kfT = [kfT_all[:, t * L:(t + 1) * L] for t in range(NH2)]
        qfT_all = P.tile([128, NH2 * L], BF16, tag="qfT", name="qfT")
        qfT = [qfT_all[:, t * L:(t + 1) * L] for t in range(NH2)]
        aTbig = P.tile([128, NH2 * L], BF16, tag="aT", name="aT")
        aT_all = [aTbig[:, t * L:(t + 1) * L] for t in range(NH2)]
        aTv = aTbig.rearrange("p (t l) -> p t l", l=L)
        # NP prefix outer products: NPs[j] = sum_{j'<=j} kf_j'^T v'_j',
        # laid out [128 (hh*64+f), NH2*F (t,d)]
        pnb = [P.tile([128, NH2 * F], BF16, tag=f"pnb{j}", name=f"pnb{j}")
               for j in range(LT - 1)]
        NPs = [P.tile([128, NH2 * F], BF16, tag=f"NP{j}", name=f"NP{j}")
               for j in range(1, LT - 1)]
        NP = [pnb[0]] + NPs  # NP[j] = prefix through block j

        # persistent K1 accumulator (2 banks)
        ka = ps.tile([128, 512], F32, tag="acc", bufs=2, name="ka")
        kb = ps.tile([128, 256], F32, tag="acc", bufs=2, name="kb")

        # ---------------- feature stage ----------------
        def qkv_mm(qk, lt, with_pd):
            """s[l, cols] = x @ Ws via fp8 DoubleRow; returns (sA, sB, pd)."""
            sA = pst([128, 512])
            sB = pst([128, 256])
            pd = psts([128, 16]) if with_pd else None
            c0 = qk * 768
            if not zb:
                nc.tensor.matmul(sA, ones1, bs_rows[qk:qk + 1, 0:512],
                                 start=True, stop=False,
                                 skip_group_check=True)
                nc.tensor.matmul(sB, ones1, bs_rows[qk:qk + 1, 512:768],
                                 start=True, stop=False,
                                 skip_group_check=True)
                if with_pd:
                    nc.tensor.matmul(pd, ones1, bs_rows[1:2, 1536:1552],
                                     start=True, stop=False,
                                     skip_group_check=True)
            for p in range(PR):
                st0 = (p == 0) and zb
                sp = (p == PR - 1)
                lhs = x8v[:, p, :, lt * 128:(lt + 1) * 128]
                nc.tensor.matmul(sA, lhs, wqk8v[:, p, :, c0:c0 + 512],
                                 start=st0, stop=sp, perf_mode=DR,
                                 skip_group_check=True)
                nc.tensor.matmul(sB, lhs, wqk8v[:, p, :, c0 + 512:c0 + 768],
                                 start=st0, stop=sp, perf_mode=DR,
                                 skip_group_check=True)
                if with_pd:
                    nc.tensor.matmul(pd, lhs, wsum8v[:, p, :, :],
                                     start=st0, stop=sp, perf_mode=DR,
                                     skip_group_check=True)
            return sA, sB, pd

        def kstage(lt):
            sA, sB, pd = qkv_mm(1, lt, True)
            dst = kf[lt]
            # kf_raw = exp(s) (scale 1/64 un-does the fp8 weight scaling)
            nc.scalar.activation(dst[:, 0:512], sA, AF.Exp, scale=IS)
            nc.scalar.activation(dst[:, 512:768], sB, AF.Exp, scale=IS)
            # r = rowmax(kf_raw) = exp(m);  fac = exp(-diag)/r
            r = sm_p.tile([128, 1], F32, tag="r", name="r")
            nc.vector.reduce_max(r, dst, axis=mybir.AxisListType.X)
            fac = sm_p.tile([128, 12], F32, tag="fac", name="fac")
            # diag = 0.5 * pd/64  ->  exp(-pd/128)
            nc.scalar.activation(fac, pd[:, 0:12], AF.Exp, scale=-0.5 * IS)
            rr = sm_p.tile([128, 1], F32, tag="rr", name="rr")
            with nc.allow_low_precision(reason="recip of exp(max), O(1)"):
                nc.vector.reciprocal(rr, r)
            facb = sm_p.tile([128, 12], BF16, tag="facb", name="facb")
            nc.vector.tensor_mul(facb, fac, rr.to_broadcast((128, 12)))
            # kf = kf_raw * fac (per head broadcast)
            nc.vector.tensor_mul(
                dst.rearrange("p (h f) -> p h f", f=F),
                dst.rearrange("p (h f) -> p h f", f=F),
                facb.to_broadcast((128, 12, F)))
            for t in range(NH2):
                nc.sync.dma_start(
                    out=kfT[t][:, lt * 128:(lt + 1) * 128],
                    in_=dst[:, t * 128:(t + 1) * 128], transpose=True)

        def vstage(lt):
            pv1 = pst([128, 512])
            pv2 = pst([128, 256])
            for p in range(PR):
                st0 = p == 0
                sp = p == PR - 1
                lhs = x8v[:, p, :, lt * 128:(lt + 1) * 128]
                nc.tensor.matmul(pv1, lhs, wv8v[:, p, :, 0:512],
                                 start=st0, stop=sp, perf_mode=DR,
                                 skip_group_check=True)
                nc.tensor.matmul(pv2, lhs, wv8v[:, p, :, 512:768],
                                 start=st0, stop=sp, perf_mode=DR,
                                 skip_group_check=True)
            # v' = 64*v kept scaled; un-scaled via wo/64 host fold
            if zb:
                nc.scalar.copy(v_p[lt][:, 0:512], pv1)
                nc.scalar.copy(v_p[lt][:, 512:768], pv2)
            else:
                # v' = psum + 64*b_v  (b_vv host-prescaled by 64)
                nc.vector.tensor_add(v_p[lt][:, 0:512], pv1, b_vv[:, 0:512])
                nc.vector.tensor_add(v_p[lt][:, 512:768], pv2,
                                     b_vv[:, 512:768])

        def njstage(lt):
            # N_lt[f, (t,d)] = kf_lt^T v'_lt per head, hh packed on partitions
            pn = pst([128, NH2 * F])
            for t in range(NH2):
                for hh in range(2):
                    h = 2 * t + hh
                    nc.tensor.matmul(
                        pn[hh * 64:hh * 64 + 64, t * F:(t + 1) * F],
                        kf[lt][:, h * F:(h + 1) * F],
                        v_p[lt][:, h * F:(h + 1) * F],
                        start=True, stop=True, skip_group_check=True)
            nc.scalar.copy(pnb[lt], pn)

        # ---------------- q stage (features + denominator) ----------------
        def qstage_mm(i):
            return qkv_mm(0, i, False)

        def qstage_exp(i, sA, sB):
            nc.scalar.activation(qf[i][:, 0:512], sA, AF.Exp, scale=IS)
            nc.scalar.activation(qf[i][:, 512:768], sB, AF.Exp, scale=IS)

        def k1stage(i):
            # ka/kb accumulate K1 for block i: add strict-lower of block i-1
            # (completing its full sum), then masked-diag of block i.
            if i > 0:
                nc.tensor.matmul(ka, maskl, kf[i - 1][:, 0:512],
                                 start=False, stop=False,
                                 skip_group_check=True)
                nc.tensor.matmul(kb, maskl, kf[i - 1][:, 512:768],
                                 start=False, stop=False,
                                 skip_group_check=True)
            nc.tensor.matmul(ka, maskd, kf[i][:, 0:512],
                             start=(i == 0), stop=(i == LT - 1),
                             skip_group_check=True)
            nc.tensor.matmul(kb, maskd, kf[i][:, 512:768],
                             start=(i == 0), stop=(i == LT - 1),
                             skip_group_check=True)

        def denstage(i):
            dn = dn_p.tile([128, H * F], BF16, tag="dn", name="dn")
            nc.vector.tensor_mul(dn[:, 0:512], qf[i][:, 0:512], ka)
            nc.vector.tensor_mul(dn[:, 512:768], qf[i][:, 512:768], kb)
            den = sm_p.tile([128, 12], F32, tag="den", name="den")
            nc.vector.reduce_sum(den, dn.rearrange("p (h f) -> p h f", f=F),
                                 axis=mybir.AxisListType.X)
            nc.vector.tensor_scalar_add(den, den, EPS)
            rq = sm_p.tile([128, 12], F32, tag="rq", name="rq")
            with nc.allow_low_precision(reason="recip of O(100) denom"):
                nc.vector.reciprocal(rq, den)
            rqb = sm_p.tile([128, 12], BF16, tag="rqb", name="rqb")
            nc.vector.tensor_copy(rqb, rq)
            nc.vector.tensor_mul(
                qf_b[i].rearrange("p (h f) -> p h f", f=F),
                qf[i].rearrange("p (h f) -> p h f", f=F),
                rqb.to_broadcast((128, 12, F)))

        def qtstage(i):
            for t in range(NH2):
                nc.sync.dma_start(
                    out=qfT[t][:, i * 128:(i + 1) * 128],
                    in_=qf_b[i][:, t * 128:(t + 1) * 128], transpose=True)

        # ---------------- attention + output projection ----------------
        def scores(i, tp, on_dve):
            """Diag-block scores for head pair-of-pairs tp (t=2tp, 2tp+1...).

            Actually per t (head pair): two 64-contraction matmuls into one
            [128, 256] psum, masked into st bf16."""
            t = tp
            pq = psts([128, 256])
            for hh in range(2):
                nc.tensor.matmul(
                    pq[:, hh * 128:(hh + 1) * 128],
                    kfT[t][hh * 64:hh * 64 + 64, i * 128:(i + 1) * 128],
                    qfT[t][hh * 64:hh * 64 + 64, i * 128:(i + 1) * 128],
                    start=True, stop=True, skip_group_check=True)
            st = st_p.tile([128, 256], BF16, tag="st", name="st")
            if on_dve:
                nc.vector.tensor_mul(st, pq, maskf)
            else:
                raw = st_p.tile([128, 256], BF16, tag="straw", name="straw")
                nc.scalar.copy(raw, pq)
                nc.gpsimd.tensor_mul(st, raw, maskf)
            return st

        def pa_pair(i, tp, sts, pas):
            """Attention for t = 2tp, 2tp+1 into one [128,256] psum."""
            pa = psts([128, 256])
            for k in range(2):
                t = 2 * tp + k
                st = sts[k]
                for hh in range(2):
                    h = 2 * t + hh
                    dst = pa[hh * 64:hh * 64 + 64, k * 128:(k + 1) * 128]
                    if i > 0:
                        nc.tensor.matmul(
                            dst,
                            NP[i - 1][hh * 64:hh * 64 + 64,
                                      t * F:(t + 1) * F],
                            qfT[t][hh * 64:hh * 64 + 64,
                                   i * 128:(i + 1) * 128],
                            start=True, stop=False, skip_group_check=True)
                    nc.tensor.matmul(
                        dst, v_p[i][:, h * F:(h + 1) * F],
                        st[:, hh * 128:(hh + 1) * 128],
                        start=(i == 0), stop=True, skip_group_check=True)
            pas.append((tp, pa))

        def aT_evac(i, tp, pa):
            nc.scalar.copy(
                aTv[:, 2 * tp:2 * tp + 2, i * 128:(i + 1) * 128],
                pa.rearrange("p (t l) -> p t l", l=128))

        def outproj(i, tp, po1, po2):
            for k in range(2):
                tt = 2 * tp + k
                st0 = zb and tt == 0
                sp = tt == NH2 - 1
                nc.tensor.matmul(po1, aT_all[tt][:, i * 128:(i + 1) * 128],
                                 wo[:, tt * 768:tt * 768 + 512],
                                 start=st0, stop=sp, skip_group_check=True)
                nc.tensor.matmul(po2, aT_all[tt][:, i * 128:(i + 1) * 128],
                                 wo[:, tt * 768 + 512:tt * 768 + 768],
                                 start=st0, stop=sp, skip_group_check=True)

        # ================= emission =================
        for lt in range(LT):
            kstage(lt)
        for lt in range(LT):
            vstage(lt)
            if lt < LT - 1:
                njstage(lt)
        # NP prefix sums on Pool (SBUF bf16)
        nc.gpsimd.tensor_add(NP[1], NP[0], pnb[1])
        nc.gpsimd.tensor_add(NP[2], NP[1], pnb[2])

        # software-pipelined i loop
        sAB = qstage_mm(0)
        qstage_exp(0, sAB[0], sAB[1])
        k1stage(0)
        denstage(0)
        qtstage(0)

        for i in range(LT):
            nxt = i + 1
            if nxt < LT:
                sAB = qstage_mm(nxt)
                qstage_exp(nxt, sAB[0], sAB[1])
            po1 = pst([128, 512])
            po2 = pst([128, 256])
            if not zb:
                nc.tensor.matmul(po1, ones1, b_orow[0:1, 0:512],
                                 start=True, stop=False,
                                 skip_group_check=True)
                nc.tensor.matmul(po2, ones1, b_orow[0:1, 512:768],
                                 start=True, stop=False,
                                 skip_group_check=True)
            pas = []
            sts01 = [scores(i, 0, True), scores(i, 1, False)]
            pa_pair(i, 0, sts01, pas)
            if nxt < LT:
                k1stage(nxt)
            sts23 = [scores(i, 2, True), scores(i, 3, False)]
            pa_pair(i, 1, sts23, pas)
            if nxt < LT:
                denstage(nxt)
            sts45 = [scores(i, 4, True), scores(i, 5, False)]
            pa_pair(i, 2, sts45, pas)
            if nxt < LT:
                qtstage(nxt)
            for tp, pa in pas:
                aT_evac(i, tp, pa)
                outproj(i, tp, po1, po2)
            # output: evac + DMA (DVE/Act split to minimize the tail)
            osb = osb_p.tile([128, E], F32, tag="osb", name="osb")
            nc.vector.tensor_copy(osb[:, 0:512], po1)
            nc.scalar.copy(osb[:, 512:768], po2)
            nc.sync.dma_start(
                out=out_d[i * 128:(i + 1) * 128, 0:512],
                in_=osb[:, 0:512])
            nc.sync.dma_start(
                out=out_d[i * 128:(i + 1) * 128, 512:768],
                in_=osb[:, 512:768])

    if fix_waits:
        _fix_waits(nc)
    return nc


_CACHE = {}


def _host_consts():
    import ml_dtypes
    bf = ml_dtypes.bfloat16
    tri = np.triu(np.ones((128, 128), dtype=np.float32))
    return {
        "mask_diag": tri.astype(bf),
        "mask_low": np.tril(np.ones((128, 128), dtype=np.float32),
                            -1).astype(bf),
        "mask_f": np.tile(tri, (1, 2)).astype(bf),
    }


def _pair_pack(w, cols):
    """[768, cols] -> [128, PR*2*cols] fp8 e-pair/plane-major layout."""
    import ml_dtypes
    f8 = ml_dtypes.float8_e4m3
    return np.ascontiguousarray(
        w.reshape(PR, 2, 128, cols).transpose(2, 0, 1, 3)
        .reshape(128, PR * 2 * cols)).astype(f8)


def _in_maps(x, w_inp, b_inp, w_out, b_out, omega):
    import ml_dtypes
    bf = ml_dtypes.bfloat16
    f = lambda a: np.ascontiguousarray(np.asarray(a), dtype=np.float32)
    x, w_inp, b_inp = f(x), f(w_inp), f(b_inp)
    w_out, b_out, omega = f(w_out), f(b_out), f(omega)
    w = w_inp[0]  # [E, 3E]
    omt = (omega.T * (float(Dh) ** -0.25)).astype(np.float64)   # [d, f]
    # fold omega into the q/k projections: Ws[:, (qk,h,f)] per head
    ws = np.empty((E, 1536), np.float64)
    wqk_full = w[:, 0:1536].astype(np.float64)
    for qk in range(2):
        for h in range(H):
            c = qk * 768 + h * 64
            ws[:, c:c + 64] = wqk_full[:, c:c + 64] @ omt
    # k-side per-head column sums (diag), padded 12->16
    wsum_full = np.zeros((E, 16), np.float64)
    wsum_full[:, 0:12] = ws[:, 768:1536].reshape(E, 12, 64).sum(axis=2)
    wqk8 = _pair_pack((ws * W8SCALE).astype(np.float32), 1536)
    wsum8 = _pair_pack((wsum_full * W8SCALE).astype(np.float32), 16)
    wv8 = _pair_pack(w[:, 1536:2304] * W8SCALE, 768)
    # wo/64 un-does the v'=64v scaling
    wo = np.ascontiguousarray(
        (w_out[0] * IS).reshape(ET, 128, 768).transpose(1, 0, 2)
        .reshape(128, ET * 768)).astype(bf)
    zb = bool(np.all(b_inp == 0.0) and np.all(b_out == 0.0))
    consts = _host_consts()
    maps = []
    for c in range(B):
        x8 = _pair_pack(x[c].T, L)
        m = {"x8": x8, "wqk8": wqk8, "wv8": wv8, "wo": wo, "wsum8": wsum8}
        if not zb:
            bs = np.zeros((2, 1536 + 16), np.float32)
            for qk in range(2):
                bq = b_inp[qk * 768:(qk + 1) * 768].astype(np.float64)
                bsh = np.empty((768,), np.float64)
                for h in range(H):
                    bsh[h * 64:(h + 1) * 64] = bq[h * 64:(h + 1) * 64] @ omt
                # bias rows feed the scaled psum: multiply by 64
                bs[qk, 0:768] = (bsh * W8SCALE).astype(np.float32)
                if qk == 1:
                    bs[1, 1536:1548] = (
                        bsh.reshape(12, 64).sum(axis=1) * W8SCALE
                    ).astype(np.float32)
            m["bs_rows"] = bs
            m["ones1"] = np.ones((1, 128), np.float32)
            m["b_vv"] = np.ascontiguousarray(np.broadcast_to(
                b_inp[1536:2304] * W8SCALE, (128, E))).astype(np.float32)
            m["b_orow"] = np.ascontiguousarray(b_out).reshape(1, E)
        m.update(consts)
        maps.append(m)
    return maps


def kernel(x, w_inp, b_inp, w_out, b_out, omega):
    maps = _in_maps(x, w_inp, b_inp, w_out, b_out, omega)
    zb = "b_vv" not in maps[0]
    key = f"nc{int(zb)}"
    if key not in _CACHE:
        _CACHE[key] = build_nc(zb=zb)
    nc = _CACHE[key]
    res = bass_utils.run_bass_kernel_spmd(nc, maps, core_ids=list(range(B)))
    return np.stack([res.results[c]["out"] for c in range(B)])


# revision 24
# speedup vs baseline: 1.0784x; 1.0390x over previous
"""Trainium2 Bass kernel v3: FAVOR (Performer) causal linear attention block.

Per batch element (data-parallel over 8 NeuronCores):
  c = x @ w_inp + b_inp; q,k,v = split(c)
  qf/kf = rfm_softmax(q/k, omega)             (FAVOR random feature maps)
  a     = causal_linear_attention(qf, kf, v)  (prefix outer-products + masked
                                               diagonal blocks)
  out   = a @ w_out + b_out

v3 design notes:
  - x transposed on host; QKV-feature and V GEMMs run as fp8e4 DoubleRow
    matmuls (2 k-planes per instruction, 0.5 cyc/row); weights pre-scaled
    by 64 into fp8 normal range, un-scaled via exp(s/64) activation scale
    (features) and wo/64 host fold (v path: v'=64v carried through).
  - q-side normalizer exp(-diag-m)/sqrt(F) cancels in a/denom: qf = exp(s_q).
  - k-side max taken as r = rowmax(exp(s_k)) on the bf16 feature tile;
    per-head factor applied as one broadcast DVE multiply.
  - K1 (cumulative kf sums) accumulated in a persistent PSUM pair via
    triu/strict-tril masks: 2 matmuls per block after the first.
  - attention: per-block diag scores (masked on DVE/Pool) + prefix NP
    outer-product matmuls; aT feeds output projection directly as lhsT.
"""

import numpy as np
from contextlib import ExitStack

import concourse.bass as bass
import concourse.tile as tile
from concourse import mybir
from concourse import bass_utils
import bass_rust

F32 = mybir.dt.float32
F32R = mybir.dt.float32r
BF16 = mybir.dt.bfloat16
F8 = mybir.dt.float8e4
AF = mybir.ActivationFunctionType
ALU = mybir.AluOpType
DR = mybir.MatmulPerfMode.DoubleRow

B, L, E, H, Dh, F = 8, 512, 768, 12, 64, 64
LT = L // 128       # 4 l-chunks
ET = E // 128       # 6 e-chunks
PR = ET // 2        # 3 e-pair chunks (DoubleRow planes)
NH2 = H // 2        # 6 head pairs
EPS = 1e-6
W8SCALE = 64.0
IS = 1.0 / W8SCALE

PHASES = []         # (name, first_instruction_number) markers for profiling


def _fix_waits(nc, cap=1):
    """Walrus codegen allows a single sync-wait per instruction; hoist excess
    waits onto injected same-engine NoOps placed directly before the offender
    (no reordering, deadlock-free)."""
    n = 0
    for fn in nc.m.functions:
        for bb in fn.blocks:
            insts = bb.instructions
            i = 0
            while i < len(insts):
                inst = insts[i]
                si = inst.sync_info
                if si is not None:
                    ow = list(si.on_wait)
                    if len(ow) > cap:
                        excess, keep = ow[:-cap], ow[-cap:]
                        si.on_wait = keep
                        for w in excess:
                            n += 1
                            nop = bass_rust.InstNoOp(
                                name=f"waitnop_{n}",
                                engine=inst.engine,
                                sync_info=bass_rust.SyncInfo(
                                    on_wait=[w], on_update=[]),
                            )
                            insts.insert(i, nop)
                            i += 1
                i += 1
    return n


def build_nc(fix_waits=True, zb=True):
    nc = bass.Bass("TRN2", target_bir_lowering=False, debug=False,
                   num_devices=8)
    PHASES.clear()

    def mark(name):
        PHASES.append((name, int(nc.get_next_instruction_name()[2:])))

    x8_d = nc.dram_tensor("x8", [128, PR * 2 * L], F8,
                          kind="ExternalInput").ap()
    xb_d = nc.dram_tensor("xb", [128, ET * L], BF16,
                          kind="ExternalInput").ap()
    wqk8_d = nc.dram_tensor("wqk8", [128, PR * 2 * 1536], F8,
                            kind="ExternalInput").ap()
    wsumb_d = nc.dram_tensor("wsumb", [128, ET * 16], BF16,
                             kind="ExternalInput").ap()
    wvb_d = nc.dram_tensor("wvb", [128, ET * 768], BF16,
                           kind="ExternalInput").ap()
    wv8_d = nc.dram_tensor("wv8", [128, PR * 2 * 768], F8,
                           kind="ExternalInput").ap()
    wo_d = nc.dram_tensor("wo", [128, ET * 768], BF16,
                          kind="ExternalInput").ap()
    masks_d = nc.dram_tensor("masks", [128, 512], BF16,
                             kind="ExternalInput").ap()
    if not zb:
        ones1_d = nc.dram_tensor("ones1", [1, 128], F32R,
                                 kind="ExternalInput").ap()
        bs_d = nc.dram_tensor("bs_rows", [2, 1536 + 16], F32R,
                              kind="ExternalInput").ap()
        b_vv_d = nc.dram_tensor("b_vv", [128, E], F32,
                                kind="ExternalInput").ap()
        b_orow_d = nc.dram_tensor("b_orow", [1, E], F32R,
                                  kind="ExternalInput").ap()
    out_d = nc.dram_tensor("out", [L, E], F32, kind="ExternalOutput").ap()

    with tile.TileContext(nc) as tc, ExitStack() as ctx:
        P = ctx.enter_context(tc.tile_pool(name="persist", bufs=1))
        st_p = ctx.enter_context(tc.tile_pool(name="stp", bufs=6))
        sm_p = ctx.enter_context(tc.tile_pool(name="smp", bufs=8))
        dn_p = ctx.enter_context(tc.tile_pool(name="dnp", bufs=2))
        osb_p = ctx.enter_context(tc.tile_pool(name="osb", bufs=2))
        ps = ctx.enter_context(tc.tile_pool(name="ps", bufs=1, space="PSUM"))

        cnt = [0]

        def pst(shape, dtype=F32, tag="big", bufs=4):
            cnt[0] += 1
            return ps.tile(shape, dtype, tag=tag, bufs=bufs,
                           name=f"pst{cnt[0]}")

        def psts(shape, dtype=F32):
            return pst(shape, dtype, tag="small", bufs=2)

        # PSUM budget: tag big x4 + small x2 + acc x2 = 8 banks.

        # Act-table warmup: absorb the 1.3us activation table load at t=0
        warm = P.tile([128, 1], F32, tag="warm", name="warm")
        nc.gpsimd.memset(warm, 0.0)
        nc.scalar.activation(warm, warm, AF.Exp)

        # ---------------- DMAs ----------------
        # SP queue: the critical-path fp8 operands, chunked by e-pair so the
        # first QKV matmuls start as early as possible.
        x8 = P.tile([128, PR * 2 * L], F8, tag="x8", name="x8")
        x8v = x8.rearrange("p (pr two l) -> p pr two l", two=2, l=L)
        wqk8 = P.tile([128, PR * 2 * 1536], F8, tag="wqk8", name="wqk8")
        wqk8v = wqk8.rearrange("p (pr two c) -> p pr two c", two=2, c=1536)
        for p in range(PR):
            nc.sync.dma_start(out=x8[:, p * 1024:(p + 1) * 1024],
                              in_=x8_d[:, p * 1024:(p + 1) * 1024])
            nc.sync.dma_start(out=wqk8[:, p * 3072:(p + 1) * 3072],
                              in_=wqk8_d[:, p * 3072:(p + 1) * 3072])
        # Act queue: the bf16 x copy + diag weights (needed by pd ~4us in,
        # after the Act warmup's table load)
        wsumb = P.tile([128, ET * 16], BF16, tag="wsumb", name="wsumb")
        xb = P.tile([128, ET * L], BF16, tag="xb", name="xb")
        xbv = xb.rearrange("p (et l) -> p et l", l=L)
        for p in range(PR):
            nc.scalar.dma_start(out=xb[:, p * 1024:(p + 1) * 1024],
                                in_=xb_d[:, p * 1024:(p + 1) * 1024])
        nc.scalar.dma_start(out=wsumb, in_=wsumb_d)
        if not zb:
            ones1 = P.tile([1, 128], F32R, tag="ones1", name="ones1")
            nc.sync.dma_start(out=ones1, in_=ones1_d)
            bs_rows = P.tile([2, 1536 + 16], F32R, tag="bs_rows",
                             name="bs_rows")
            nc.sync.dma_start(out=bs_rows, in_=bs_d)

        # Pool (SWDGE) queue: few big DMAs (SWDGE prep ~1us each serializes
        # the queue) in need order: masks, wvb, wv8, wo.
        masks = P.tile([128, 512], BF16, tag="masks", name="masks")
        nc.gpsimd.dma_start(out=masks, in_=masks_d)
        maskd = masks[:, 0:128]
        maskl = masks[:, 128:256]
        maskf = masks[:, 256:512]
        wvb = P.tile([128, ET * 768], BF16, tag="wvb", name="wvb")
        nc.gpsimd.dma_start(out=wvb, in_=wvb_d)
        wv8 = P.tile([128, PR * 2 * 768], F8, tag="wv8", name="wv8")
        wv8v = wv8.rearrange("p (pr two c) -> p pr two c", two=2, c=768)
        nc.gpsimd.dma_start(out=wv8, in_=wv8_d)
        wo = P.tile([128, ET * 768], BF16, tag="wo", name="wo")
        nc.gpsimd.dma_start(out=wo, in_=wo_d)
        if not zb:
            b_vv = P.tile([128, E], F32, tag="b_vv", name="b_vv")
            nc.gpsimd.dma_start(out=b_vv, in_=b_vv_d)
            b_orow = P.tile([1, E], F32R, tag="b_orow", name="b_orow")
            nc.gpsimd.dma_start(out=b_orow, in_=b_orow_d)

        # ---------------- persistent SBUF tiles ----------------
        kf = [P.tile([128, H * F], BF16, tag=f"kf{lt}", name=f"kf{lt}")
              for lt in range(LT)]
        qf = [P.tile([128, H * F], BF16, tag=f"qf{lt}", name=f"qf{lt}")
              for lt in range(LT)]
        qf_b = [P.tile([128, H * F], BF16, tag=f"qfb{lt}", name=f"qfb{lt}")
                for lt in range(LT)]
        v_p = [P.tile([128, E], BF16, tag=f"vp{lt}", name=f"vp{lt}")
               for lt in range(LT)]
        kfT_all = P.tile([128, NH2 * L], BF16, tag="kfT", name="kfT")
        kfT = [kfT_all[:, t * L:(t + 1) * L] for t in range(NH2)]
        qfT_all = P.tile([128, NH2 * L], BF16, tag="qfT", name="qfT")
        qfT = [qfT_all[:, t * L:(t + 1) * L] for t in range(NH2)]
        aTbig = P.tile([128, NH2 * L], BF16, tag="aT", name="aT")
        aT_all = [aTbig[:, t * L:(t + 1) * L] for t in range(NH2)]
        aTv = aTbig.rearrange("p (t l) -> p t l", l=L)
        # NP prefix outer products: NPs[j] = sum_{j'<=j} kf_j'^T v'_j',
        # laid out [128 (hh*64+f), NH2*F (t,d)]
        pnb = [P.tile([128, NH2 * F], BF16, tag=f"pnb{j}", name=f"pnb{j}")
               for j in range(LT - 1)]
        NPs = [P.tile([128, NH2 * F], BF16, tag=f"NP{j}", name=f"NP{j}")
               for j in range(1, LT - 1)]
        NP = [pnb[0]] + NPs  # NP[j] = prefix through block j

        # persistent K1 accumulator (2 banks)
        ka = ps.tile([128, 512], F32, tag="acc", bufs=2, name="ka")
        kb = ps.tile([128, 256], F32, tag="acc", bufs=2, name="kb")

        # ---------------- feature stage ----------------
        def qkv_mm(qk, lt, with_pd=False):
            """s[l, cols] = x @ Ws via fp8 DoubleRow; returns (sA, sB, _)."""
            sA = pst([128, 512])
            sB = pst([128, 256])
            c0 = qk * 768
            if not zb:
                nc.tensor.matmul(sA, ones1, bs_rows[qk:qk + 1, 0:512],
                                 start=True, stop=False,
                                 skip_group_check=True)
                nc.tensor.matmul(sB, ones1, bs_rows[qk:qk + 1, 512:768],
                                 start=True, stop=False,
                                 skip_group_check=True)
            for p in range(PR):
                st0 = (p == 0) and zb
                sp = (p == PR - 1)
                lhs = x8v[:, p, :, lt * 128:(lt + 1) * 128]
                nc.tensor.matmul(sA, lhs, wqk8v[:, p, :, c0:c0 + 512],
                                 start=st0, stop=sp, perf_mode=DR,
                                 skip_group_check=True)
                nc.tensor.matmul(sB, lhs, wqk8v[:, p, :, c0 + 512:c0 + 768],
                                 start=st0, stop=sp, perf_mode=DR,
                                 skip_group_check=True)
            return sA, sB, None

        def kstage_mm(lt):
            """fp8 feature matmuls + exp; pd deferred (waits on the slower
            bf16 x load) so it doesn't block the PE queue."""
            sA, sB, _ = qkv_mm(1, lt, False)
            dst = kf[lt]
            # kf_raw = exp(s) (scale 1/64 un-does the fp8 weight scaling)
            nc.scalar.activation(dst[:, 0:512], sA, AF.Exp, scale=IS)
            nc.scalar.activation(dst[:, 512:768], sB, AF.Exp, scale=IS)

        def kstage_fac(lt):
            dst = kf[lt]
            pd = psts([128, 16])
            if not zb:
                nc.tensor.matmul(pd, ones1, bs_rows[1:2, 1536:1552],
                                 start=True, stop=False,
                                 skip_group_check=True)
            for et in range(ET):
                nc.tensor.matmul(pd, xbv[:, et, lt * 128:(lt + 1) * 128],
                                 wsumb[:, et * 16:(et + 1) * 16],
                                 start=(et == 0) and zb, stop=(et == ET - 1),
                                 skip_group_check=True)
            # r = rowmax(kf_raw) = exp(m);  fac = exp(-diag)/r
            r = sm_p.tile([128, 1], F32, tag="r", name="r")
            nc.vector.reduce_max(r, dst, axis=mybir.AxisListType.X)
            fac = sm_p.tile([128, 12], F32, tag="fac", name="fac")
            # diag = 0.5 * pd (pd unscaled bf16 path)  ->  exp(-pd/2)
            nc.scalar.activation(fac, pd[:, 0:12], AF.Exp, scale=-0.5)
            rr = sm_p.tile([128, 1], F32, tag="rr", name="rr")
            with nc.allow_low_precision(reason="recip of exp(max), O(1)"):
                nc.vector.reciprocal(rr, r)
            facb = sm_p.tile([128, 12], BF16, tag="facb", name="facb")
            nc.vector.tensor_mul(facb, fac, rr.to_broadcast((128, 12)))
            # kf = kf_raw * fac (per head broadcast)
            nc.vector.tensor_mul(
                dst.rearrange("p (h f) -> p h f", f=F),
                dst.rearrange("p (h f) -> p h f", f=F),
                facb.to_broadcast((128, 12, F)))
            for t in range(NH2):
                nc.sync.dma_start(
                    out=kfT[t][:, lt * 128:(lt + 1) * 128],
                    in_=dst[:, t * 128:(t + 1) * 128], transpose=True)

        def vstage(lt):
            """v' = 64*v. Block 0 runs bf16 (low-support early positions
            see v errors unaveraged); later blocks run fp8 DoubleRow."""
            pv1 = pst([128, 512])
            pv2 = pst([128, 256])
            if lt == 0:
                for et in range(ET):
                    st0 = et == 0
                    sp = et == ET - 1
                    lhs = xbv[:, et, lt * 128:(lt + 1) * 128]
                    nc.tensor.matmul(pv1, lhs,
                                     wvb[:, et * 768:et * 768 + 512],
                                     start=st0, stop=sp,
                                     skip_group_check=True)
                    nc.tensor.matmul(pv2, lhs,
                                     wvb[:, et * 768 + 512:(et + 1) * 768],
                                     start=st0, stop=sp,
                                     skip_group_check=True)
            else:
                for p in range(PR):
                    st0 = p == 0
                    sp = p == PR - 1
                    lhs = x8v[:, p, :, lt * 128:(lt + 1) * 128]
                    nc.tensor.matmul(pv1, lhs, wv8v[:, p, :, 0:512],
                                     start=st0, stop=sp, perf_mode=DR,
                                     skip_group_check=True)
                    nc.tensor.matmul(pv2, lhs, wv8v[:, p, :, 512:768],
                                     start=st0, stop=sp, perf_mode=DR,
                                     skip_group_check=True)
            # v' = 64*v kept scaled; un-scaled via wo/64 host fold
            if zb:
                nc.scalar.copy(v_p[lt][:, 0:512], pv1)
                nc.scalar.copy(v_p[lt][:, 512:768], pv2)
            else:
                # v' = psum + 64*b_v  (b_vv host-prescaled by 64)
                nc.vector.tensor_add(v_p[lt][:, 0:512], pv1, b_vv[:, 0:512])
                nc.vector.tensor_add(v_p[lt][:, 512:768], pv2,
                                     b_vv[:, 512:768])

        def njstage(lt):
            # N_lt[f, (t,d)] = kf_lt^T v'_lt per head, hh packed on partitions
            pn = pst([128, NH2 * F])
            for t in range(NH2):
                for hh in range(2):
                    h = 2 * t + hh
                    nc.tensor.matmul(
                        pn[hh * 64:hh * 64 + 64, t * F:(t + 1) * F],
                        kf[lt][:, h * F:(h + 1) * F],
                        v_p[lt][:, h * F:(h + 1) * F],
                        start=True, stop=True, skip_group_check=True)
            nc.scalar.copy(pnb[lt], pn)

        # ---------------- q stage (features + denominator) ----------------
        def qstage_mm(i):
            return qkv_mm(0, i, False)

        def qstage_exp(i, sA, sB):
            nc.scalar.activation(qf[i][:, 0:512], sA, AF.Exp, scale=IS)
            nc.scalar.activation(qf[i][:, 512:768], sB, AF.Exp, scale=IS)

        def k1stage(i):
            # ka/kb accumulate K1 for block i: add strict-lower of block i-1
            # (completing its full sum), then masked-diag of block i.
            if i > 0:
                nc.tensor.matmul(ka, maskl, kf[i - 1][:, 0:512],
                                 start=False, stop=False,
                                 skip_group_check=True)
                nc.tensor.matmul(kb, maskl, kf[i - 1][:, 512:768],
                                 start=False, stop=False,
                                 skip_group_check=True)
            nc.tensor.matmul(ka, maskd, kf[i][:, 0:512],
                             start=(i == 0), stop=(i == LT - 1),
                             skip_group_check=True)
            nc.tensor.matmul(kb, maskd, kf[i][:, 512:768],
                             start=(i == 0), stop=(i == LT - 1),
                             skip_group_check=True)

        def denstage(i):
            dn = dn_p.tile([128, H * F], BF16, tag="dn", name="dn")
            nc.vector.tensor_mul(dn[:, 0:512], qf[i][:, 0:512], ka)
            nc.vector.tensor_mul(dn[:, 512:768], qf[i][:, 512:768], kb)
            den = sm_p.tile([128, 12], F32, tag="den", name="den")
            nc.vector.reduce_sum(den, dn.rearrange("p (h f) -> p h f", f=F),
                                 axis=mybir.AxisListType.X)
            nc.vector.tensor_scalar_add(den, den, EPS)
            rq = sm_p.tile([128, 12], F32, tag="rq", name="rq")
            with nc.allow_low_precision(reason="recip of O(100) denom"):
                nc.vector.reciprocal(rq, den)
            rqb = sm_p.tile([128, 12], BF16, tag="rqb", name="rqb")
            nc.vector.tensor_copy(rqb, rq)
            nc.vector.tensor_mul(
                qf_b[i].rearrange("p (h f) -> p h f", f=F),
                qf[i].rearrange("p (h f) -> p h f", f=F),
                rqb.to_broadcast((128, 12, F)))

        def qtstage(i):
            for t in range(NH2):
                nc.sync.dma_start(
                    out=qfT[t][:, i * 128:(i + 1) * 128],
                    in_=qf_b[i][:, t * 128:(t + 1) * 128], transpose=True)

        # ---------------- attention + output projection ----------------
        def scores(i, tp, on_dve):
            """Diag-block scores for head pair-of-pairs tp (t=2tp, 2tp+1...).

            Actually per t (head pair): two 64-contraction matmuls into one
            [128, 256] psum, masked into st bf16."""
            t = tp
            pq = psts([128, 256])
            for hh in range(2):
                nc.tensor.matmul(
                    pq[:, hh * 128:(hh + 1) * 128],
                    kfT[t][hh * 64:hh * 64 + 64, i * 128:(i + 1) * 128],
                    qfT[t][hh * 64:hh * 64 + 64, i * 128:(i + 1) * 128],
                    start=True, stop=True, skip_group_check=True)
            st = st_p.tile([128, 256], BF16, tag="st", name="st")
            if on_dve:
                nc.vector.tensor_mul(st, pq, maskf)
            else:
                raw = st_p.tile([128, 256], BF16, tag="straw", name="straw")
                nc.scalar.copy(raw, pq)
                nc.gpsimd.tensor_mul(st, raw, maskf)
            return st

        def pa_pair(i, tp, sts, pas):
            """Attention for t = 2tp, 2tp+1 into one [128,256] psum."""
            pa = psts([128, 256])
            for k in range(2):
                t = 2 * tp + k
                st = sts[k]
                for hh in range(2):
                    h = 2 * t + hh
                    dst = pa[hh * 64:hh * 64 + 64, k * 128:(k + 1) * 128]
                    if i > 0:
                        nc.tensor.matmul(
                            dst,
                            NP[i - 1][hh * 64:hh * 64 + 64,
                                      t * F:(t + 1) * F],
                            qfT[t][hh * 64:hh * 64 + 64,
                                   i * 128:(i + 1) * 128],
                            start=True, stop=False, skip_group_check=True)
                    nc.tensor.matmul(
                        dst, v_p[i][:, h * F:(h + 1) * F],
                        st[:, hh * 128:(hh + 1) * 128],
                        start=(i == 0), stop=True, skip_group_check=True)
            pas.append((tp, pa))

        def aT_evac(i, tp, pa):
            nc.scalar.copy(
                aTv[:, 2 * tp:2 * tp + 2, i * 128:(i + 1) * 128],
                pa.rearrange("p (t l) -> p t l", l=128))

        def outproj(i, tp, po1, po2):
            for k in range(2):
                tt = 2 * tp + k
                st0 = zb and tt == 0
                sp = tt == NH2 - 1
                nc.tensor.matmul(po1, aT_all[tt][:, i * 128:(i + 1) * 128],
                                 wo[:, tt * 768:tt * 768 + 512],
                                 start=st0, stop=sp, skip_group_check=True)
                nc.tensor.matmul(po2, aT_all[tt][:, i * 128:(i + 1) * 128],
                                 wo[:, tt * 768 + 512:tt * 768 + 768],
                                 start=st0, stop=sp, skip_group_check=True)

        def iblk(i, extra=()):
            """Attention + output projection for block i; `extra` stages are
            interleaved to fill engine slack."""
            extra = list(extra)
            po1 = pst([128, 512])
            po2 = pst([128, 256])
            if not zb:
                nc.tensor.matmul(po1, ones1, b_orow[0:1, 0:512],
                                 start=True, stop=False,
                                 skip_group_check=True)
                nc.tensor.matmul(po2, ones1, b_orow[0:1, 512:768],
                                 start=True, stop=False,
                                 skip_group_check=True)
            pas = []
            sts01 = [scores(i, 0, True), scores(i, 1, False)]
            if extra:
                extra.pop(0)()
            pa_pair(i, 0, sts01, pas)
            aT_evac(i, 0, pas[0][1])
            outproj(i, 0, po1, po2)
            sts23 = [scores(i, 2, True), scores(i, 3, False)]
            if extra:
                extra.pop(0)()
            pa_pair(i, 1, sts23, pas)
            aT_evac(i, 1, pas[1][1])
            outproj(i, 1, po1, po2)
            sts45 = [scores(i, 4, True), scores(i, 5, False)]
            if extra:
                extra.pop(0)()
            pa_pair(i, 2, sts45, pas)
            aT_evac(i, 2, pas[2][1])
            outproj(i, 2, po1, po2)
            # output: evac + DMA (DVE/Act split to minimize the tail)
            osb = osb_p.tile([128, E], F32, tag="osb", name="osb")
            nc.vector.tensor_copy(osb[:, 0:512], po1)
            nc.scalar.copy(osb[:, 512:768], po2)
            nc.sync.dma_start(
                out=out_d[i * 128:(i + 1) * 128, 0:512],
                in_=osb[:, 0:512])
            nc.sync.dma_start(
                out=out_d[i * 128:(i + 1) * 128, 512:768],
                in_=osb[:, 512:768])
            for fn in extra:
                fn()

        def qstage(j):
            sA, sB, _ = qkv_mm(0, j)
            qstage_exp(j, sA, sB)

        def bstage(j):
            k1stage(j)
            denstage(j)
            qtstage(j)

        # ================= emission =================
        # Priority spine: k features -> q features -> K1/den/qfT chains,
        # with v projections and attention blocks filling in behind.
        mark("kstage0")
        for lt in range(LT):
            kstage_mm(lt)
        mark("qstage0")
        qstage(0)
        mark("kfac01")
        kstage_fac(0)
        kstage_fac(1)
        mark("bstage0")
        bstage(0)
        mark("kfac23")
        kstage_fac(2)
        kstage_fac(3)
        mark("qstage1")
        qstage(1)
        mark("bstage1")
        bstage(1)
        mark("vstage0")
        vstage(0)
        njstage(0)
        mark("qstage2")
        qstage(2)
        mark("vstage1")
        vstage(1)
        njstage(1)
        nc.gpsimd.tensor_add(NP[1], NP[0], pnb[1])

        def ex_v2():
            vstage(2)
            njstage(2)
            nc.gpsimd.tensor_add(NP[2], NP[1], pnb[2])

        def ex_v3():
            vstage(3)

        mark("iblk0")
        iblk(0, extra=[lambda: (k1stage(2), denstage(2)),
                       lambda: qtstage(2), ex_v2])
        mark("qstage3")
        qstage(3)
        mark("iblk1")
        iblk(1, extra=[lambda: (k1stage(3), denstage(3)),
                       lambda: qtstage(3), ex_v3])
        mark("iblk2")
        iblk(2)
        mark("iblk3")
        iblk(3)

    if fix_waits:
        _fix_waits(nc)
    return nc


_CACHE = {}


def _host_consts():
    import ml_dtypes
    bf = ml_dtypes.bfloat16
    tri = np.triu(np.ones((128, 128), dtype=np.float32))
    masks = np.concatenate(
        [tri, np.tril(np.ones((128, 128), dtype=np.float32), -1),
         np.tile(tri, (1, 2))], axis=1)
    return {"masks": masks.astype(bf)}


def _pair_pack(w, cols):
    """[768, cols] -> [128, PR*2*cols] fp8 e-pair/plane-major layout."""
    import ml_dtypes
    f8 = ml_dtypes.float8_e4m3
    return np.ascontiguousarray(
        w.reshape(PR, 2, 128, cols).transpose(2, 0, 1, 3)
        .reshape(128, PR * 2 * cols)).astype(f8)


def _in_maps(x, w_inp, b_inp, w_out, b_out, omega):
    import ml_dtypes
    bf = ml_dtypes.bfloat16
    f = lambda a: np.ascontiguousarray(np.asarray(a), dtype=np.float32)
    x, w_inp, b_inp = f(x), f(w_inp), f(b_inp)
    w_out, b_out, omega = f(w_out), f(b_out), f(omega)
    w = w_inp[0]  # [E, 3E]
    omt = (omega.T * (float(Dh) ** -0.25)).astype(np.float64)   # [d, f]
    # fold omega into the q/k projections: Ws[:, (qk,h,f)] per head
    ws = np.empty((E, 1536), np.float64)
    wqk_full = w[:, 0:1536].astype(np.float64)
    for qk in range(2):
        for h in range(H):
            c = qk * 768 + h * 64
            ws[:, c:c + 64] = wqk_full[:, c:c + 64] @ omt
    # k-side per-head column sums (diag), padded 12->16, bf16 et-major
    wsum_full = np.zeros((E, 16), np.float64)
    wsum_full[:, 0:12] = ws[:, 768:1536].reshape(E, 12, 64).sum(axis=2)
    wqk8 = _pair_pack((ws * W8SCALE).astype(np.float32), 1536)
    wsumb = np.ascontiguousarray(
        wsum_full.astype(np.float32).reshape(ET, 128, 16)
        .transpose(1, 0, 2).reshape(128, ET * 16)).astype(bf)
    wv8 = _pair_pack(w[:, 1536:2304] * W8SCALE, 768)
    # bf16 v weights (block 0), same x64 scale so v'=64v uniformly
    wvb = np.ascontiguousarray(
        (w[:, 1536:2304] * W8SCALE).reshape(ET, 128, 768)
        .transpose(1, 0, 2).reshape(128, ET * 768)).astype(bf)
    # wo/64 un-does the v'=64v scaling
    wo = np.ascontiguousarray(
        (w_out[0] * IS).reshape(ET, 128, 768).transpose(1, 0, 2)
        .reshape(128, ET * 768)).astype(bf)
    zb = bool(np.all(b_inp == 0.0) and np.all(b_out == 0.0))
    consts = _host_consts()
    maps = []
    for c in range(B):
        xT = x[c].T
        x8 = _pair_pack(xT, L)
        xbn = np.ascontiguousarray(
            xT.reshape(ET, 128, L).transpose(1, 0, 2)
            .reshape(128, ET * L)).astype(bf)
        m = {"x8": x8, "xb": xbn, "wqk8": wqk8, "wv8": wv8, "wvb": wvb,
             "wo": wo, "wsumb": wsumb}
        if not zb:
            bs = np.zeros((2, 1536 + 16), np.float32)
            for qk in range(2):
                bq = b_inp[qk * 768:(qk + 1) * 768].astype(np.float64)
                bsh = np.empty((768,), np.float64)
                for h in range(H):
                    bsh[h * 64:(h + 1) * 64] = bq[h * 64:(h + 1) * 64] @ omt
                # bias rows feed the x64-scaled psum: multiply by 64;
                # the pd psum is unscaled bf16: sums stay unscaled
                bs[qk, 0:768] = (bsh * W8SCALE).astype(np.float32)
                if qk == 1:
                    bs[1, 1536:1548] = (
                        bsh.reshape(12, 64).sum(axis=1)).astype(np.float32)
            m["bs_rows"] = bs
            m["ones1"] = np.ones((1, 128), np.float32)
            m["b_vv"] = np.ascontiguousarray(np.broadcast_to(
                b_inp[1536:2304] * W8SCALE, (128, E))).astype(np.float32)
            m["b_orow"] = np.ascontiguousarray(b_out).reshape(1, E)
        m.update(consts)
        maps.append(m)
    return maps


def kernel(x, w_inp, b_inp, w_out, b_out, omega):
    maps = _in_maps(x, w_inp, b_inp, w_out, b_out, omega)
    zb = "b_vv" not in maps[0]
    key = f"nc{int(zb)}"
    if key not in _CACHE:
        _CACHE[key] = build_nc(zb=zb)
    nc = _CACHE[key]
    res = bass_utils.run_bass_kernel_spmd(nc, maps, core_ids=list(range(B)))
    return np.stack([res.results[c]["out"] for c in range(B)])


# revision 33
# speedup vs baseline: 1.1002x; 1.0202x over previous
"""Trainium2 Bass kernel v3: FAVOR (Performer) causal linear attention block.

Per batch element (data-parallel over 8 NeuronCores):
  c = x @ w_inp + b_inp; q,k,v = split(c)
  qf/kf = rfm_softmax(q/k, omega)             (FAVOR random feature maps)
  a     = causal_linear_attention(qf, kf, v)  (prefix outer-products + masked
                                               diagonal blocks)
  out   = a @ w_out + b_out

v3 design notes:
  - x transposed on host; QKV-feature and V GEMMs run as fp8e4 DoubleRow
    matmuls (2 k-planes per instruction, 0.5 cyc/row); weights pre-scaled
    by 64 into fp8 normal range, un-scaled via exp(s/64) activation scale
    (features) and wo/64 host fold (v path: v'=64v carried through).
  - q-side normalizer exp(-diag-m)/sqrt(F) cancels in a/denom: qf = exp(s_q).
  - k-side max taken as r = rowmax(exp(s_k)) on the bf16 feature tile;
    per-head factor applied as one broadcast DVE multiply.
  - K1 (cumulative kf sums) accumulated in a persistent PSUM pair via
    triu/strict-tril masks: 2 matmuls per block after the first.
  - attention: per-block diag scores (masked on DVE/Pool) + prefix NP
    outer-product matmuls; aT feeds output projection directly as lhsT.
"""

import numpy as np
from contextlib import ExitStack

import concourse.bass as bass
import concourse.tile as tile
from concourse import mybir
from concourse import bass_utils
import bass_rust

F32 = mybir.dt.float32
F32R = mybir.dt.float32r
BF16 = mybir.dt.bfloat16
F8 = mybir.dt.float8e4
AF = mybir.ActivationFunctionType
ALU = mybir.AluOpType
DR = mybir.MatmulPerfMode.DoubleRow

B, L, E, H, Dh, F = 8, 512, 768, 12, 64, 64
LT = L // 128       # 4 l-chunks
ET = E // 128       # 6 e-chunks
PR = ET // 2        # 3 e-pair chunks (DoubleRow planes)
NH2 = H // 2        # 6 head pairs
EPS = 1e-6
W8SCALE = 64.0
IS = 1.0 / W8SCALE

PHASES = []         # (name, first_instruction_number) markers for profiling


def _fix_waits(nc, cap=1):
    """Walrus codegen allows a single sync-wait per instruction; hoist excess
    waits onto injected same-engine NoOps placed directly before the offender
    (no reordering, deadlock-free)."""
    n = 0
    for fn in nc.m.functions:
        for bb in fn.blocks:
            insts = bb.instructions
            i = 0
            while i < len(insts):
                inst = insts[i]
                si = inst.sync_info
                if si is not None:
                    ow = list(si.on_wait)
                    if len(ow) > cap:
                        excess, keep = ow[:-cap], ow[-cap:]
                        si.on_wait = keep
                        for w in excess:
                            n += 1
                            nop = bass_rust.InstNoOp(
                                name=f"waitnop_{n}",
                                engine=inst.engine,
                                sync_info=bass_rust.SyncInfo(
                                    on_wait=[w], on_update=[]),
                            )
                            insts.insert(i, nop)
                            i += 1
                i += 1
    return n


def build_nc(fix_waits=True, zb=True):
    nc = bass.Bass("TRN2", target_bir_lowering=False, debug=False,
                   num_devices=8)
    PHASES.clear()

    def mark(name):
        PHASES.append((name, int(nc.get_next_instruction_name()[2:])))

    x8_d = nc.dram_tensor("x8", [128, PR * 2 * L], F8,
                          kind="ExternalInput").ap()
    xb_d = nc.dram_tensor("xb", [128, ET * L], BF16,
                          kind="ExternalInput").ap()
    wqk8_d = nc.dram_tensor("wqk8", [128, PR * 2 * 1536], F8,
                            kind="ExternalInput").ap()
    wvb_d = nc.dram_tensor("wvb", [128, ET * 768], BF16,
                           kind="ExternalInput").ap()
    wv8_d = nc.dram_tensor("wv8", [128, PR * 2 * 768], F8,
                           kind="ExternalInput").ap()
    wo_d = nc.dram_tensor("wo", [128, ET * 768], BF16,
                          kind="ExternalInput").ap()
    consts_d = nc.dram_tensor("consts", [128, 512 + ET * 16], BF16,
                              kind="ExternalInput").ap()
    if not zb:
        ones1_d = nc.dram_tensor("ones1", [1, 128], F32R,
                                 kind="ExternalInput").ap()
        bs_d = nc.dram_tensor("bs_rows", [2, 1536 + 16], F32R,
                              kind="ExternalInput").ap()
        b_vv_d = nc.dram_tensor("b_vv", [128, E], F32,
                                kind="ExternalInput").ap()
        b_orow_d = nc.dram_tensor("b_orow", [1, E], F32R,
                                  kind="ExternalInput").ap()
    out_d = nc.dram_tensor("out", [L, E], F32, kind="ExternalOutput").ap()

    with tile.TileContext(nc) as tc, ExitStack() as ctx:
        P = ctx.enter_context(tc.tile_pool(name="persist", bufs=1))
        st_p = ctx.enter_context(tc.tile_pool(name="stp", bufs=6))
        sm_p = ctx.enter_context(tc.tile_pool(name="smp", bufs=8))
        dn_p = ctx.enter_context(tc.tile_pool(name="dnp", bufs=2))
        osb_p = ctx.enter_context(tc.tile_pool(name="osb", bufs=2))
        ps = ctx.enter_context(tc.tile_pool(name="ps", bufs=1, space="PSUM"))

        cnt = [0]

        def pst(shape, dtype=F32, tag="big", bufs=4):
            cnt[0] += 1
            return ps.tile(shape, dtype, tag=tag, bufs=bufs,
                           name=f"pst{cnt[0]}")

        def psts(shape, dtype=F32):
            return pst(shape, dtype, tag="small", bufs=2)

        # PSUM budget: tag big x4 + small x2 + acc x2 = 8 banks.

        # Act-table warmup: absorb the 1.3us activation table load at t=0
        warm = P.tile([128, 1], F32, tag="warm", name="warm")
        nc.gpsimd.memset(warm, 0.0)
        nc.scalar.activation(warm, warm, AF.Exp)

        # ---------------- DMAs ----------------
        # SP queue spine, in critical-path order: x8, k-side weights, bf16 x
        # (pd), q-side weights. Strided q/k-half DMAs keep transfers minimal.
        x8 = P.tile([128, PR * 2 * L], F8, tag="x8", name="x8")
        x8v = x8.rearrange("p (pr two l) -> p pr two l", two=2, l=L)
        wqk8 = P.tile([128, PR * 2 * 1536], F8, tag="wqk8", name="wqk8")
        wqk8v = wqk8.rearrange("p (pr two c) -> p pr two c", two=2, c=1536)
        wqk8dv = wqk8_d.rearrange("p (pr two c) -> p pr two c", two=2, c=1536)
        xb = P.tile([128, ET * L], BF16, tag="xb", name="xb")
        xbv = xb.rearrange("p (et l) -> p et l", l=L)
        nc.sync.dma_start(out=x8, in_=x8_d)
        nc.sync.dma_start(out=wqk8v[:, :, :, 768:1536],
                          in_=wqk8dv[:, :, :, 768:1536])
        nc.sync.dma_start(out=xb[:, 0:1024], in_=xb_d[:, 0:1024])
        nc.sync.dma_start(out=wqk8v[:, :, :, 0:768],
                          in_=wqk8dv[:, :, :, 0:768])
        nc.sync.dma_start(out=xb[:, 1024:2048], in_=xb_d[:, 1024:2048])
        nc.sync.dma_start(out=xb[:, 2048:3072], in_=xb_d[:, 2048:3072])
        if not zb:
            ones1 = P.tile([1, 128], F32R, tag="ones1", name="ones1")
            nc.sync.dma_start(out=ones1, in_=ones1_d)
            bs_rows = P.tile([2, 1536 + 16], F32R, tag="bs_rows",
                             name="bs_rows")
            nc.sync.dma_start(out=bs_rows, in_=bs_d)

        # Pool (SWDGE) queue: few big DMAs (SWDGE prep ~1us each serializes
        # the queue) in need order: masks+wsum, wvb, wo, wv8.
        consts = P.tile([128, 512 + ET * 16], BF16, tag="consts",
                        name="consts")
        nc.gpsimd.dma_start(out=consts, in_=consts_d)
        maskd = consts[:, 0:128]
        maskl = consts[:, 128:256]
        maskf = consts[:, 256:512]
        wsumb = consts[:, 512:512 + ET * 16]
        wvb = P.tile([128, ET * 768], BF16, tag="wvb", name="wvb")
        nc.gpsimd.dma_start(out=wvb, in_=wvb_d)
        wo = P.tile([128, ET * 768], BF16, tag="wo", name="wo")
        nc.gpsimd.dma_start(out=wo, in_=wo_d)
        wv8 = P.tile([128, PR * 2 * 768], F8, tag="wv8", name="wv8")
        wv8v = wv8.rearrange("p (pr two c) -> p pr two c", two=2, c=768)
        nc.gpsimd.dma_start(out=wv8, in_=wv8_d)
        if not zb:
            b_vv = P.tile([128, E], F32, tag="b_vv", name="b_vv")
            nc.gpsimd.dma_start(out=b_vv, in_=b_vv_d)
            b_orow = P.tile([1, E], F32R, tag="b_orow", name="b_orow")
            nc.gpsimd.dma_start(out=b_orow, in_=b_orow_d)

        # ---------------- persistent SBUF tiles ----------------
        kf = [P.tile([128, H * F], BF16, tag=f"kf{lt}", name=f"kf{lt}")
              for lt in range(LT)]
        qf = [P.tile([128, H * F], BF16, tag=f"qf{lt}", name=f"qf{lt}")
              for lt in range(LT)]
        qf_b = [P.tile([128, H * F], BF16, tag=f"qfb{lt}", name=f"qfb{lt}")
                for lt in range(LT)]
        v_p = [P.tile([128, E], BF16, tag=f"vp{lt}", name=f"vp{lt}")
               for lt in range(LT)]
        kfT_all = P.tile([128, NH2 * L], BF16, tag="kfT", name="kfT")
        kfT = [kfT_all[:, t * L:(t + 1) * L] for t in range(NH2)]
        qfT_all = P.tile([128, NH2 * L], BF16, tag="qfT", name="qfT")
        qfT = [qfT_all[:, t * L:(t + 1) * L] for t in range(NH2)]
        aTbig = P.tile([128, NH2 * L], BF16, tag="aT", name="aT")
        aT_all = [aTbig[:, t * L:(t + 1) * L] for t in range(NH2)]
        aTv = aTbig.rearrange("p (t l) -> p t l", l=L)
        # NP prefix outer products: NPs[j] = sum_{j'<=j} kf_j'^T v'_j',
        # laid out [128 (hh*64+f), NH2*F (t,d)]
        pnb = [P.tile([128, NH2 * F], BF16, tag=f"pnb{j}", name=f"pnb{j}")
               for j in range(LT - 1)]
        NPs = [P.tile([128, NH2 * F], BF16, tag=f"NP{j}", name=f"NP{j}")
               for j in range(1, LT - 1)]
        NP = [pnb[0]] + NPs  # NP[j] = prefix through block j

        # persistent K1 accumulator (2 banks)
        ka = ps.tile([128, 512], F32, tag="acc", bufs=2, name="ka")
        kb = ps.tile([128, 256], F32, tag="acc", bufs=2, name="kb")

        # ---------------- feature stage ----------------
        def qkv_mm(qk, lt, with_pd=False):
            """s[l, cols] = x @ Ws via fp8 DoubleRow; returns (sA, sB, _)."""
            sA = pst([128, 512])
            sB = pst([128, 256])
            c0 = qk * 768
            if not zb:
                nc.tensor.matmul(sA, ones1, bs_rows[qk:qk + 1, 0:512],
                                 start=True, stop=False,
                                 skip_group_check=True)
                nc.tensor.matmul(sB, ones1, bs_rows[qk:qk + 1, 512:768],
                                 start=True, stop=False,
                                 skip_group_check=True)
            for p in range(PR):
                st0 = (p == 0) and zb
                sp = (p == PR - 1)
                lhs = x8v[:, p, :, lt * 128:(lt + 1) * 128]
                nc.tensor.matmul(sA, lhs, wqk8v[:, p, :, c0:c0 + 512],
                                 start=st0, stop=sp, perf_mode=DR,
                                 skip_group_check=True)
                nc.tensor.matmul(sB, lhs, wqk8v[:, p, :, c0 + 512:c0 + 768],
                                 start=st0, stop=sp, perf_mode=DR,
                                 skip_group_check=True)
            return sA, sB, None

        def kstage_mm(lt):
            """fp8 feature matmuls + exp; pd deferred (waits on the slower
            bf16 x load) so it doesn't block the PE queue."""
            sA, sB, _ = qkv_mm(1, lt, False)
            dst = kf[lt]
            # kf_raw = exp(s) (scale 1/64 un-does the fp8 weight scaling)
            nc.scalar.activation(dst[:, 0:512], sA, AF.Exp, scale=IS)
            nc.scalar.activation(dst[:, 512:768], sB, AF.Exp, scale=IS)

        def kstage_fac(lt):
            dst = kf[lt]
            pd = psts([128, 16])
            if not zb:
                nc.tensor.matmul(pd, ones1, bs_rows[1:2, 1536:1552],
                                 start=True, stop=False,
                                 skip_group_check=True)
            for et in range(ET):
                nc.tensor.matmul(pd, xbv[:, et, lt * 128:(lt + 1) * 128],
                                 wsumb[:, et * 16:(et + 1) * 16],
                                 start=(et == 0) and zb, stop=(et == ET - 1),
                                 skip_group_check=True)
            # r = rowmax(kf_raw) = exp(m);  fac = exp(-diag)/r
            # odd blocks run the scale multiply on Pool to unload DVE
            heavy = nc.vector if lt % 2 == 0 else nc.gpsimd
            r = sm_p.tile([128, 1], F32, tag="r", name="r")
            nc.vector.reduce_max(r, dst, axis=mybir.AxisListType.X)
            fac = sm_p.tile([128, 12], F32, tag="fac", name="fac")
            # diag = 0.5 * pd (pd unscaled bf16 path)  ->  exp(-pd/2)
            nc.scalar.activation(fac, pd[:, 0:12], AF.Exp, scale=-0.5)
            rr = sm_p.tile([128, 1], F32, tag="rr", name="rr")
            with nc.allow_low_precision(reason="recip of exp(max), O(1)"):
                nc.vector.reciprocal(rr, r)
            facb = sm_p.tile([128, 12], BF16, tag="facb", name="facb")
            nc.vector.tensor_mul(facb, fac, rr.to_broadcast((128, 12)))
            # kf = kf_raw * fac (per head broadcast)
            heavy.tensor_mul(
                dst.rearrange("p (h f) -> p h f", f=F),
                dst.rearrange("p (h f) -> p h f", f=F),
                facb.to_broadcast((128, 12, F)))
            for t in range(NH2):
                nc.sync.dma_start(
                    out=kfT[t][:, lt * 128:(lt + 1) * 128],
                    in_=dst[:, t * 128:(t + 1) * 128], transpose=True)

        def vstage(lt):
            """v' = 64*v. Block 0 runs bf16 (low-support early positions
            see v errors unaveraged); later blocks run fp8 DoubleRow."""
            pv1 = pst([128, 512])
            pv2 = pst([128, 256])
            if lt == 0:
                for et in range(ET):
                    st0 = et == 0
                    sp = et == ET - 1
                    lhs = xbv[:, et, lt * 128:(lt + 1) * 128]
                    nc.tensor.matmul(pv1, lhs,
                                     wvb[:, et * 768:et * 768 + 512],
                                     start=st0, stop=sp,
                                     skip_group_check=True)
                    nc.tensor.matmul(pv2, lhs,
                                     wvb[:, et * 768 + 512:(et + 1) * 768],
                                     start=st0, stop=sp,
                                     skip_group_check=True)
            else:
                for p in range(PR):
                    st0 = p == 0
                    sp = p == PR - 1
                    lhs = x8v[:, p, :, lt * 128:(lt + 1) * 128]
                    nc.tensor.matmul(pv1, lhs, wv8v[:, p, :, 0:512],
                                     start=st0, stop=sp, perf_mode=DR,
                                     skip_group_check=True)
                    nc.tensor.matmul(pv2, lhs, wv8v[:, p, :, 512:768],
                                     start=st0, stop=sp, perf_mode=DR,
                                     skip_group_check=True)
            # v' = 64*v kept scaled; un-scaled via wo/64 host fold
            if zb:
                nc.scalar.copy(v_p[lt][:, 0:512], pv1)
                nc.scalar.copy(v_p[lt][:, 512:768], pv2)
            else:
                # v' = psum + 64*b_v  (b_vv host-prescaled by 64)
                nc.vector.tensor_add(v_p[lt][:, 0:512], pv1, b_vv[:, 0:512])
                nc.vector.tensor_add(v_p[lt][:, 512:768], pv2,
                                     b_vv[:, 512:768])

        def njstage(lt):
            # N_lt[f, (t,d)] = kf_lt^T v'_lt per head, hh packed on partitions
            pn = pst([128, NH2 * F])
            for t in range(NH2):
                for hh in range(2):
                    h = 2 * t + hh
                    nc.tensor.matmul(
                        pn[hh * 64:hh * 64 + 64, t * F:(t + 1) * F],
                        kf[lt][:, h * F:(h + 1) * F],
                        v_p[lt][:, h * F:(h + 1) * F],
                        start=True, stop=True, skip_group_check=True)
            nc.scalar.copy(pnb[lt], pn)

        # ---------------- q stage (features + denominator) ----------------
        def qstage_mm(i):
            return qkv_mm(0, i, False)

        def qstage_exp(i, sA, sB):
            nc.scalar.activation(qf[i][:, 0:512], sA, AF.Exp, scale=IS)
            nc.scalar.activation(qf[i][:, 512:768], sB, AF.Exp, scale=IS)

        def k1stage(i):
            # ka/kb accumulate K1 for block i: add strict-lower of block i-1
            # (completing its full sum), then masked-diag of block i.
            if i > 0:
                nc.tensor.matmul(ka, maskl, kf[i - 1][:, 0:512],
                                 start=False, stop=False,
                                 skip_group_check=True)
                nc.tensor.matmul(kb, maskl, kf[i - 1][:, 512:768],
                                 start=False, stop=False,
                                 skip_group_check=True)
            nc.tensor.matmul(ka, maskd, kf[i][:, 0:512],
                             start=(i == 0), stop=(i == LT - 1),
                             skip_group_check=True)
            nc.tensor.matmul(kb, maskd, kf[i][:, 512:768],
                             start=(i == 0), stop=(i == LT - 1),
                             skip_group_check=True)

        def denstage(i):
            # den = qf . K1 per head; rq = 1/den (EPS dropped: den >= ~3e-3)
            dn = dn_p.tile([128, H * F], BF16, tag="dn", name="dn")
            nc.vector.tensor_mul(dn[:, 0:512], qf[i][:, 0:512], ka)
            nc.vector.tensor_mul(dn[:, 512:768], qf[i][:, 512:768], kb)
            den = sm_p.tile([128, 12], F32, tag="den", name="den")
            nc.vector.reduce_sum(den, dn.rearrange("p (h f) -> p h f", f=F),
                                 axis=mybir.AxisListType.X)
            rqb = sm_p.tile([128, 12], BF16, tag="rqb", name="rqb")
            with nc.allow_low_precision(reason="recip of O(100) denom"):
                nc.vector.reciprocal(rqb, den)
            nc.vector.tensor_mul(
                qf_b[i].rearrange("p (h f) -> p h f", f=F),
                qf[i].rearrange("p (h f) -> p h f", f=F),
                rqb.to_broadcast((128, 12, F)))

        def qtstage(i):
            for t in range(NH2):
                nc.sync.dma_start(
                    out=qfT[t][:, i * 128:(i + 1) * 128],
                    in_=qf_b[i][:, t * 128:(t + 1) * 128], transpose=True)

        # ---------------- attention + output projection ----------------
        def scores(i, tp, on_dve):
            """Diag-block scores for head pair-of-pairs tp (t=2tp, 2tp+1...).

            Actually per t (head pair): two 64-contraction matmuls into one
            [128, 256] psum, masked into st bf16."""
            t = tp
            pq = psts([128, 256])
            for hh in range(2):
                nc.tensor.matmul(
                    pq[:, hh * 128:(hh + 1) * 128],
                    kfT[t][hh * 64:hh * 64 + 64, i * 128:(i + 1) * 128],
                    qfT[t][hh * 64:hh * 64 + 64, i * 128:(i + 1) * 128],
                    start=True, stop=True, skip_group_check=True)
            st = st_p.tile([128, 256], BF16, tag="st", name="st")
            if on_dve:
                nc.vector.tensor_mul(st, pq, maskf)
            else:
                raw = st_p.tile([128, 256], BF16, tag="straw", name="straw")
                nc.scalar.copy(raw, pq)
                nc.gpsimd.tensor_mul(st, raw, maskf)
            return st

        def pa_pair(i, tp, sts, pas):
            """Attention for t = 2tp, 2tp+1 into one [128,256] psum."""
            pa = psts([128, 256])
            for k in range(2):
                t = 2 * tp + k
                st = sts[k]
                for hh in range(2):
                    h = 2 * t + hh
                    dst = pa[hh * 64:hh * 64 + 64, k * 128:(k + 1) * 128]
                    if i > 0:
                        nc.tensor.matmul(
                            dst,
                            NP[i - 1][hh * 64:hh * 64 + 64,
                                      t * F:(t + 1) * F],
                            qfT[t][hh * 64:hh * 64 + 64,
                                   i * 128:(i + 1) * 128],
                            start=True, stop=False, skip_group_check=True)
                    nc.tensor.matmul(
                        dst, v_p[i][:, h * F:(h + 1) * F],
                        st[:, hh * 128:(hh + 1) * 128],
                        start=(i == 0), stop=True, skip_group_check=True)
            pas.append((tp, pa))

        def aT_evac(i, tp, pa):
            nc.scalar.copy(
                aTv[:, 2 * tp:2 * tp + 2, i * 128:(i + 1) * 128],
                pa.rearrange("p (t l) -> p t l", l=128))

        def outproj(i, tp, po1, po2):
            for k in range(2):
                tt = 2 * tp + k
                st0 = zb and tt == 0
                sp = tt == NH2 - 1
                nc.tensor.matmul(po1, aT_all[tt][:, i * 128:(i + 1) * 128],
                                 wo[:, tt * 768:tt * 768 + 512],
                                 start=st0, stop=sp, skip_group_check=True)
                nc.tensor.matmul(po2, aT_all[tt][:, i * 128:(i + 1) * 128],
                                 wo[:, tt * 768 + 512:tt * 768 + 768],
                                 start=st0, stop=sp, skip_group_check=True)

        def iblk(i, extra=()):
            """Attention + output projection for block i; `extra` stages are
            interleaved to fill engine slack."""
            extra = list(extra)
            po1 = pst([128, 512])
            po2 = pst([128, 256])
            if not zb:
                nc.tensor.matmul(po1, ones1, b_orow[0:1, 0:512],
                                 start=True, stop=False,
                                 skip_group_check=True)
                nc.tensor.matmul(po2, ones1, b_orow[0:1, 512:768],
                                 start=True, stop=False,
                                 skip_group_check=True)
            pas = []
            sts01 = [scores(i, 0, True), scores(i, 1, False)]
            if extra:
                extra.pop(0)()
            pa_pair(i, 0, sts01, pas)
            aT_evac(i, 0, pas[0][1])
            outproj(i, 0, po1, po2)
            sts23 = [scores(i, 2, True), scores(i, 3, False)]
            if extra:
                extra.pop(0)()
            pa_pair(i, 1, sts23, pas)
            aT_evac(i, 1, pas[1][1])
            outproj(i, 1, po1, po2)
            sts45 = [scores(i, 4, True), scores(i, 5, False)]
            if extra:
                extra.pop(0)()
            pa_pair(i, 2, sts45, pas)
            aT_evac(i, 2, pas[2][1])
            outproj(i, 2, po1, po2)
            # output: evac + DMA (DVE/Act split to minimize the tail)
            osb = osb_p.tile([128, E], F32, tag="osb", name="osb")
            nc.vector.tensor_copy(osb[:, 0:512], po1)
            nc.scalar.copy(osb[:, 512:768], po2)
            nc.sync.dma_start(
                out=out_d[i * 128:(i + 1) * 128, 0:512],
                in_=osb[:, 0:512])
            nc.sync.dma_start(
                out=out_d[i * 128:(i + 1) * 128, 512:768],
                in_=osb[:, 512:768])
            for fn in extra:
                fn()

        def qstage(j):
            sA, sB, _ = qkv_mm(0, j)
            qstage_exp(j, sA, sB)

        def bstage(j):
            k1stage(j)
            denstage(j)
            qtstage(j)

        # ================= emission =================
        # Priority spine: k features -> q features -> K1/den/qfT chains,
        # with v projections and attention blocks filling in behind.
        mark("kstage0")
        kstage_mm(0)
        mark("qstage0")
        qstage(0)
        mark("kmm123")
        kstage_mm(1)
        kstage_mm(2)
        kstage_mm(3)
        mark("kfac0")
        kstage_fac(0)
        mark("bstage0")
        bstage(0)
        mark("kfac1")
        kstage_fac(1)
        mark("qstage1")
        qstage(1)
        mark("bstage1")
        bstage(1)
        mark("kfac23")
        kstage_fac(2)
        kstage_fac(3)
        mark("vstage0")
        vstage(0)
        njstage(0)
        mark("qstage2")
        qstage(2)
        mark("vstage1")
        vstage(1)
        njstage(1)
        nc.gpsimd.tensor_add(NP[1], NP[0], pnb[1])

        def ex_v2():
            vstage(2)
            njstage(2)
            nc.gpsimd.tensor_add(NP[2], NP[1], pnb[2])

        def ex_v3():
            vstage(3)

        mark("iblk0")
        iblk(0, extra=[lambda: (k1stage(2), denstage(2)),
                       lambda: qtstage(2), ex_v2])
        mark("qstage3")
        qstage(3)
        mark("iblk1")
        iblk(1, extra=[lambda: (k1stage(3), denstage(3)),
                       lambda: qtstage(3), ex_v3])
        mark("iblk2")
        iblk(2)
        mark("iblk3")
        iblk(3)

    if fix_waits:
        _fix_waits(nc)
    return nc


_CACHE = {}


def _host_consts(wsumb):
    import ml_dtypes
    bf = ml_dtypes.bfloat16
    tri = np.triu(np.ones((128, 128), dtype=np.float32))
    masks = np.concatenate(
        [tri, np.tril(np.ones((128, 128), dtype=np.float32), -1),
         np.tile(tri, (1, 2))], axis=1).astype(bf)
    return {"consts": np.concatenate([masks, wsumb], axis=1)}


def _pair_pack(w, cols):
    """[768, cols] -> [128, PR*2*cols] fp8 e-pair/plane-major layout."""
    import ml_dtypes
    f8 = ml_dtypes.float8_e4m3
    return np.ascontiguousarray(
        w.reshape(PR, 2, 128, cols).transpose(2, 0, 1, 3)
        .reshape(128, PR * 2 * cols)).astype(f8)


def _in_maps(x, w_inp, b_inp, w_out, b_out, omega):
    import ml_dtypes
    bf = ml_dtypes.bfloat16
    f = lambda a: np.ascontiguousarray(np.asarray(a), dtype=np.float32)
    x, w_inp, b_inp = f(x), f(w_inp), f(b_inp)
    w_out, b_out, omega = f(w_out), f(b_out), f(omega)
    w = w_inp[0]  # [E, 3E]
    omt = (omega.T * (float(Dh) ** -0.25)).astype(np.float64)   # [d, f]
    # fold omega into the q/k projections: Ws[:, (qk,h,f)] per head
    ws = np.empty((E, 1536), np.float64)
    wqk_full = w[:, 0:1536].astype(np.float64)
    for qk in range(2):
        for h in range(H):
            c = qk * 768 + h * 64
            ws[:, c:c + 64] = wqk_full[:, c:c + 64] @ omt
    # k-side per-head column sums (diag), padded 12->16, bf16 et-major
    wsum_full = np.zeros((E, 16), np.float64)
    wsum_full[:, 0:12] = ws[:, 768:1536].reshape(E, 12, 64).sum(axis=2)
    wqk8 = _pair_pack((ws * W8SCALE).astype(np.float32), 1536)
    wsumb = np.ascontiguousarray(
        wsum_full.astype(np.float32).reshape(ET, 128, 16)
        .transpose(1, 0, 2).reshape(128, ET * 16)).astype(bf)
    consts = _host_consts(wsumb)
    wv8 = _pair_pack(w[:, 1536:2304] * W8SCALE, 768)
    # bf16 v weights (block 0), same x64 scale so v'=64v uniformly
    wvb = np.ascontiguousarray(
        (w[:, 1536:2304] * W8SCALE).reshape(ET, 128, 768)
        .transpose(1, 0, 2).reshape(128, ET * 768)).astype(bf)
    # wo/64 un-does the v'=64v scaling
    wo = np.ascontiguousarray(
        (w_out[0] * IS).reshape(ET, 128, 768).transpose(1, 0, 2)
        .reshape(128, ET * 768)).astype(bf)
    zb = bool(np.all(b_inp == 0.0) and np.all(b_out == 0.0))
    maps = []
    for c in range(B):
        xT = x[c].T
        x8 = _pair_pack(xT, L)
        xbn = np.ascontiguousarray(
            xT.reshape(ET, 128, L).transpose(1, 0, 2)
            .reshape(128, ET * L)).astype(bf)
        m = {"x8": x8, "xb": xbn, "wqk8": wqk8, "wv8": wv8, "wvb": wvb,
             "wo": wo}
        if not zb:
            bs = np.zeros((2, 1536 + 16), np.float32)
            for qk in range(2):
                bq = b_inp[qk * 768:(qk + 1) * 768].astype(np.float64)
                bsh = np.empty((768,), np.float64)
                for h in range(H):
                    bsh[h * 64:(h + 1) * 64] = bq[h * 64:(h + 1) * 64] @ omt
                # bias rows feed the x64-scaled psum: multiply by 64;
                # the pd psum is unscaled bf16: sums stay unscaled
                bs[qk, 0:768] = (bsh * W8SCALE).astype(np.float32)
                if qk == 1:
                    bs[1, 1536:1548] = (
                        bsh.reshape(12, 64).sum(axis=1)).astype(np.float32)
            m["bs_rows"] = bs
            m["ones1"] = np.ones((1, 128), np.float32)
            m["b_vv"] = np.ascontiguousarray(np.broadcast_to(
                b_inp[1536:2304] * W8SCALE, (128, E))).astype(np.float32)
            m["b_orow"] = np.ascontiguousarray(b_out).reshape(1, E)
        m.update(consts)
        maps.append(m)
    return maps


def kernel(x, w_inp, b_inp, w_out, b_out, omega):
    maps = _in_maps(x, w_inp, b_inp, w_out, b_out, omega)
    zb = "b_vv" not in maps[0]
    key = f"nc{int(zb)}"
    if key not in _CACHE:
        _CACHE[key] = build_nc(zb=zb)
    nc = _CACHE[key]
    res = bass_utils.run_bass_kernel_spmd(nc, maps, core_ids=list(range(B)))
    return np.stack([res.results[c]["out"] for c in range(B)])


# revision 35
# speedup vs baseline: 1.1510x; 1.0461x over previous
"""Trainium2 Bass kernel v3: FAVOR (Performer) causal linear attention block.

Per batch element (data-parallel over 8 NeuronCores):
  c = x @ w_inp + b_inp; q,k,v = split(c)
  qf/kf = rfm_softmax(q/k, omega)             (FAVOR random feature maps)
  a     = causal_linear_attention(qf, kf, v)  (prefix outer-products + masked
                                               diagonal blocks)
  out   = a @ w_out + b_out

v3 design notes:
  - x transposed on host; QKV-feature and V GEMMs run as fp8e4 DoubleRow
    matmuls (2 k-planes per instruction, 0.5 cyc/row); weights pre-scaled
    by 64 into fp8 normal range, un-scaled via exp(s/64) activation scale
    (features) and wo/64 host fold (v path: v'=64v carried through).
  - q-side normalizer exp(-diag-m)/sqrt(F) cancels in a/denom: qf = exp(s_q).
  - k-side max taken as r = rowmax(exp(s_k)) on the bf16 feature tile;
    per-head factor applied as one broadcast DVE multiply.
  - K1 (cumulative kf sums) accumulated in a persistent PSUM pair via
    triu/strict-tril masks: 2 matmuls per block after the first.
  - attention: per-block diag scores (masked on DVE/Pool) + prefix NP
    outer-product matmuls; aT feeds output projection directly as lhsT.
"""

import numpy as np
from contextlib import ExitStack

import concourse.bass as bass
import concourse.tile as tile
from concourse import mybir
from concourse import bass_utils
import bass_rust

F32 = mybir.dt.float32
F32R = mybir.dt.float32r
BF16 = mybir.dt.bfloat16
F8 = mybir.dt.float8e4
AF = mybir.ActivationFunctionType
ALU = mybir.AluOpType
DR = mybir.MatmulPerfMode.DoubleRow

B, L, E, H, Dh, F = 8, 512, 768, 12, 64, 64
LT = L // 128       # 4 l-chunks
ET = E // 128       # 6 e-chunks
PR = ET // 2        # 3 e-pair chunks (DoubleRow planes)
NH2 = H // 2        # 6 head pairs
EPS = 1e-6
W8SCALE = 64.0
IS = 1.0 / W8SCALE

PHASES = []         # (name, first_instruction_number) markers for profiling


def _fix_waits(nc, cap=1):
    """Walrus codegen allows a single sync-wait per instruction; hoist excess
    waits onto injected same-engine NoOps placed directly before the offender
    (no reordering, deadlock-free)."""
    n = 0
    for fn in nc.m.functions:
        for bb in fn.blocks:
            insts = bb.instructions
            i = 0
            while i < len(insts):
                inst = insts[i]
                si = inst.sync_info
                if si is not None:
                    ow = list(si.on_wait)
                    if len(ow) > cap:
                        excess, keep = ow[:-cap], ow[-cap:]
                        si.on_wait = keep
                        for w in excess:
                            n += 1
                            nop = bass_rust.InstNoOp(
                                name=f"waitnop_{n}",
                                engine=inst.engine,
                                sync_info=bass_rust.SyncInfo(
                                    on_wait=[w], on_update=[]),
                            )
                            insts.insert(i, nop)
                            i += 1
                i += 1
    return n


def build_nc(fix_waits=True, zb=True):
    nc = bass.Bass("TRN2", target_bir_lowering=False, debug=False,
                   num_devices=8)
    PHASES.clear()

    def mark(name):
        PHASES.append((name, int(nc.get_next_instruction_name()[2:])))

    x8_d = nc.dram_tensor("x8", [128, PR * 2 * L], F8,
                          kind="ExternalInput").ap()
    xb_d = nc.dram_tensor("xb", [128, ET * L], BF16,
                          kind="ExternalInput").ap()
    wqk8_d = nc.dram_tensor("wqk8", [128, PR * 2 * 1536], F8,
                            kind="ExternalInput").ap()
    wvb_d = nc.dram_tensor("wvb", [128, ET * 768], BF16,
                           kind="ExternalInput").ap()
    wv8_d = nc.dram_tensor("wv8", [128, PR * 2 * 768], F8,
                           kind="ExternalInput").ap()
    wo_d = nc.dram_tensor("wo", [128, ET * 768], BF16,
                          kind="ExternalInput").ap()
    consts_d = nc.dram_tensor("consts", [128, 512 + ET * 16], BF16,
                              kind="ExternalInput").ap()
    if not zb:
        ones1_d = nc.dram_tensor("ones1", [1, 128], F32R,
                                 kind="ExternalInput").ap()
        bs_d = nc.dram_tensor("bs_rows", [2, 1536 + 16], F32R,
                              kind="ExternalInput").ap()
        b_vv_d = nc.dram_tensor("b_vv", [128, E], F32,
                                kind="ExternalInput").ap()
        b_orow_d = nc.dram_tensor("b_orow", [1, E], F32R,
                                  kind="ExternalInput").ap()
    out_d = nc.dram_tensor("out", [L, E], F32, kind="ExternalOutput").ap()

    with tile.TileContext(nc) as tc, ExitStack() as ctx:
        P = ctx.enter_context(tc.tile_pool(name="persist", bufs=1))
        st_p = ctx.enter_context(tc.tile_pool(name="stp", bufs=6))
        sm_p = ctx.enter_context(tc.tile_pool(name="smp", bufs=8))
        dn_p = ctx.enter_context(tc.tile_pool(name="dnp", bufs=2))
        osb_p = ctx.enter_context(tc.tile_pool(name="osb", bufs=2))
        ps = ctx.enter_context(tc.tile_pool(name="ps", bufs=1, space="PSUM"))

        cnt = [0]

        def pst(shape, dtype=F32, tag="big", bufs=4):
            cnt[0] += 1
            return ps.tile(shape, dtype, tag=tag, bufs=bufs,
                           name=f"pst{cnt[0]}")

        def psts(shape, dtype=F32):
            return pst(shape, dtype, tag="small", bufs=2)

        # PSUM budget: tag big x4 + small x2 + acc x2 = 8 banks.

        # Act-table warmup: absorb the 1.3us activation table load at t=0
        warm = P.tile([128, 1], F32, tag="warm", name="warm")
        nc.gpsimd.memset(warm, 0.0)
        nc.scalar.activation(warm, warm, AF.Exp)

        # ---------------- DMAs ----------------
        # SP queue spine, in critical-path order: x8, k-side weights, bf16 x
        # (pd), q-side weights. Strided q/k-half DMAs keep transfers minimal.
        x8 = P.tile([128, PR * 2 * L], F8, tag="x8", name="x8")
        x8v = x8.rearrange("p (pr two l) -> p pr two l", two=2, l=L)
        wqk8 = P.tile([128, PR * 2 * 1536], F8, tag="wqk8", name="wqk8")
        wqk8v = wqk8.rearrange("p (pr two c) -> p pr two c", two=2, c=1536)
        wqk8dv = wqk8_d.rearrange("p (pr two c) -> p pr two c", two=2, c=1536)
        xb = P.tile([128, ET * L], BF16, tag="xb", name="xb")
        xbv = xb.rearrange("p (et l) -> p et l", l=L)
        nc.sync.dma_start(out=x8, in_=x8_d)
        nc.sync.dma_start(out=wqk8v[:, :, :, 768:1536],
                          in_=wqk8dv[:, :, :, 768:1536])
        nc.sync.dma_start(out=xb, in_=xb_d)
        nc.sync.dma_start(out=wqk8v[:, :, :, 0:768],
                          in_=wqk8dv[:, :, :, 0:768])
        if not zb:
            ones1 = P.tile([1, 128], F32R, tag="ones1", name="ones1")
            nc.sync.dma_start(out=ones1, in_=ones1_d)
            bs_rows = P.tile([2, 1536 + 16], F32R, tag="bs_rows",
                             name="bs_rows")
            nc.sync.dma_start(out=bs_rows, in_=bs_d)

        # Pool (SWDGE) queue: few big DMAs (SWDGE prep ~1us each serializes
        # the queue) in need order: masks+wsum, wvb, wo, wv8.
        consts = P.tile([128, 512 + ET * 16], BF16, tag="consts",
                        name="consts")
        nc.gpsimd.dma_start(out=consts, in_=consts_d)
        maskd = consts[:, 0:128]
        maskl = consts[:, 128:256]
        maskf = consts[:, 256:512]
        wsumb = consts[:, 512:512 + ET * 16]
        wvb = P.tile([128, ET * 768], BF16, tag="wvb", name="wvb")
        nc.gpsimd.dma_start(out=wvb, in_=wvb_d)
        wo = P.tile([128, ET * 768], BF16, tag="wo", name="wo")
        nc.gpsimd.dma_start(out=wo, in_=wo_d)
        wv8 = P.tile([128, PR * 2 * 768], F8, tag="wv8", name="wv8")
        wv8v = wv8.rearrange("p (pr two c) -> p pr two c", two=2, c=768)
        nc.gpsimd.dma_start(out=wv8, in_=wv8_d)
        if not zb:
            b_vv = P.tile([128, E], F32, tag="b_vv", name="b_vv")
            nc.gpsimd.dma_start(out=b_vv, in_=b_vv_d)
            b_orow = P.tile([1, E], F32R, tag="b_orow", name="b_orow")
            nc.gpsimd.dma_start(out=b_orow, in_=b_orow_d)

        # ---------------- persistent SBUF tiles ----------------
        kf = [P.tile([128, H * F], BF16, tag=f"kf{lt}", name=f"kf{lt}")
              for lt in range(LT)]
        qf = [P.tile([128, H * F], BF16, tag=f"qf{lt}", name=f"qf{lt}")
              for lt in range(LT)]
        qf_b = [P.tile([128, H * F], BF16, tag=f"qfb{lt}", name=f"qfb{lt}")
                for lt in range(LT)]
        v_p = [P.tile([128, E], BF16, tag=f"vp{lt}", name=f"vp{lt}")
               for lt in range(LT)]
        kfT_all = P.tile([128, NH2 * L], BF16, tag="kfT", name="kfT")
        kfT = [kfT_all[:, t * L:(t + 1) * L] for t in range(NH2)]
        qfT_all = P.tile([128, NH2 * L], BF16, tag="qfT", name="qfT")
        qfT = [qfT_all[:, t * L:(t + 1) * L] for t in range(NH2)]
        aTbig = P.tile([128, NH2 * L], BF16, tag="aT", name="aT")
        aT_all = [aTbig[:, t * L:(t + 1) * L] for t in range(NH2)]
        aTv = aTbig.rearrange("p (t l) -> p t l", l=L)
        # NP prefix outer products: NPs[j] = sum_{j'<=j} kf_j'^T v'_j',
        # laid out [128 (hh*64+f), NH2*F (t,d)]
        pnb = [P.tile([128, NH2 * F], BF16, tag=f"pnb{j}", name=f"pnb{j}")
               for j in range(LT - 1)]
        NPs = [P.tile([128, NH2 * F], BF16, tag=f"NP{j}", name=f"NP{j}")
               for j in range(1, LT - 1)]
        NP = [pnb[0]] + NPs  # NP[j] = prefix through block j

        # persistent K1 accumulator (2 banks)
        ka = ps.tile([128, 512], F32, tag="acc", bufs=2, name="ka")
        kb = ps.tile([128, 256], F32, tag="acc", bufs=2, name="kb")

        # ---------------- feature stage ----------------
        def qkv_mm(qk, lt, with_pd=False):
            """s[l, cols] = x @ Ws via fp8 DoubleRow; returns (sA, sB, _)."""
            sA = pst([128, 512])
            sB = pst([128, 256])
            c0 = qk * 768
            if not zb:
                nc.tensor.matmul(sA, ones1, bs_rows[qk:qk + 1, 0:512],
                                 start=True, stop=False,
                                 skip_group_check=True)
                nc.tensor.matmul(sB, ones1, bs_rows[qk:qk + 1, 512:768],
                                 start=True, stop=False,
                                 skip_group_check=True)
            for p in range(PR):
                st0 = (p == 0) and zb
                sp = (p == PR - 1)
                lhs = x8v[:, p, :, lt * 128:(lt + 1) * 128]
                nc.tensor.matmul(sA, lhs, wqk8v[:, p, :, c0:c0 + 512],
                                 start=st0, stop=sp, perf_mode=DR,
                                 skip_group_check=True)
                nc.tensor.matmul(sB, lhs, wqk8v[:, p, :, c0 + 512:c0 + 768],
                                 start=st0, stop=sp, perf_mode=DR,
                                 skip_group_check=True)
            return sA, sB, None

        def kstage_mm(lt):
            """fp8 feature matmuls + exp; pd deferred (waits on the slower
            bf16 x load) so it doesn't block the PE queue."""
            sA, sB, _ = qkv_mm(1, lt, False)
            dst = kf[lt]
            # kf_raw = exp(s) (scale 1/64 un-does the fp8 weight scaling)
            nc.scalar.activation(dst[:, 0:512], sA, AF.Exp, scale=IS)
            nc.scalar.activation(dst[:, 512:768], sB, AF.Exp, scale=IS)

        def kstage_fac(lt):
            dst = kf[lt]
            pd = psts([128, 16])
            if not zb:
                nc.tensor.matmul(pd, ones1, bs_rows[1:2, 1536:1552],
                                 start=True, stop=False,
                                 skip_group_check=True)
            for et in range(ET):
                nc.tensor.matmul(pd, xbv[:, et, lt * 128:(lt + 1) * 128],
                                 wsumb[:, et * 16:(et + 1) * 16],
                                 start=(et == 0) and zb, stop=(et == ET - 1),
                                 skip_group_check=True)
            # r = rowmax(kf_raw) = exp(m);  fac = exp(-diag)/r
            # odd blocks run the scale multiply on Pool to unload DVE
            heavy = nc.vector if lt % 2 == 0 else nc.gpsimd
            r = sm_p.tile([128, 1], F32, tag="r", name="r")
            nc.vector.reduce_max(r, dst, axis=mybir.AxisListType.X)
            fac = sm_p.tile([128, 12], F32, tag="fac", name="fac")
            # diag = 0.5 * pd (pd unscaled bf16 path)  ->  exp(-pd/2)
            nc.scalar.activation(fac, pd[:, 0:12], AF.Exp, scale=-0.5)
            rr = sm_p.tile([128, 1], F32, tag="rr", name="rr")
            with nc.allow_low_precision(reason="recip of exp(max), O(1)"):
                nc.vector.reciprocal(rr, r)
            facb = sm_p.tile([128, 12], BF16, tag="facb", name="facb")
            nc.vector.tensor_mul(facb, fac, rr.to_broadcast((128, 12)))
            # kf = kf_raw * fac (per head broadcast)
            heavy.tensor_mul(
                dst.rearrange("p (h f) -> p h f", f=F),
                dst.rearrange("p (h f) -> p h f", f=F),
                facb.to_broadcast((128, 12, F)))
            for t in range(NH2):
                nc.sync.dma_start(
                    out=kfT[t][:, lt * 128:(lt + 1) * 128],
                    in_=dst[:, t * 128:(t + 1) * 128], transpose=True)

        def vstage(lt):
            """v' = 64*v. Block 0 runs bf16 (low-support early positions
            see v errors unaveraged); later blocks run fp8 DoubleRow."""
            pv1 = pst([128, 512])
            pv2 = pst([128, 256])
            if lt == 0:
                for et in range(ET):
                    st0 = et == 0
                    sp = et == ET - 1
                    lhs = xbv[:, et, lt * 128:(lt + 1) * 128]
                    nc.tensor.matmul(pv1, lhs,
                                     wvb[:, et * 768:et * 768 + 512],
                                     start=st0, stop=sp,
                                     skip_group_check=True)
                    nc.tensor.matmul(pv2, lhs,
                                     wvb[:, et * 768 + 512:(et + 1) * 768],
                                     start=st0, stop=sp,
                                     skip_group_check=True)
            else:
                for p in range(PR):
                    st0 = p == 0
                    sp = p == PR - 1
                    lhs = x8v[:, p, :, lt * 128:(lt + 1) * 128]
                    nc.tensor.matmul(pv1, lhs, wv8v[:, p, :, 0:512],
                                     start=st0, stop=sp, perf_mode=DR,
                                     skip_group_check=True)
                    nc.tensor.matmul(pv2, lhs, wv8v[:, p, :, 512:768],
                                     start=st0, stop=sp, perf_mode=DR,
                                     skip_group_check=True)
            # v' = 64*v kept scaled; un-scaled via wo/64 host fold
            if zb:
                nc.scalar.copy(v_p[lt][:, 0:512], pv1)
                nc.scalar.copy(v_p[lt][:, 512:768], pv2)
            else:
                # v' = psum + 64*b_v  (b_vv host-prescaled by 64)
                nc.vector.tensor_add(v_p[lt][:, 0:512], pv1, b_vv[:, 0:512])
                nc.vector.tensor_add(v_p[lt][:, 512:768], pv2,
                                     b_vv[:, 512:768])

        def njstage(lt):
            # N_lt[f, (t,d)] = kf_lt^T v'_lt per head, hh packed on partitions
            pn = pst([128, NH2 * F])
            for t in range(NH2):
                for hh in range(2):
                    h = 2 * t + hh
                    nc.tensor.matmul(
                        pn[hh * 64:hh * 64 + 64, t * F:(t + 1) * F],
                        kf[lt][:, h * F:(h + 1) * F],
                        v_p[lt][:, h * F:(h + 1) * F],
                        start=True, stop=True, skip_group_check=True)
            nc.scalar.copy(pnb[lt], pn)

        # ---------------- q stage (features + denominator) ----------------
        def qstage_mm(i):
            return qkv_mm(0, i, False)

        def qstage_exp(i, sA, sB):
            nc.scalar.activation(qf[i][:, 0:512], sA, AF.Exp, scale=IS)
            nc.scalar.activation(qf[i][:, 512:768], sB, AF.Exp, scale=IS)

        def k1stage(i):
            # ka/kb accumulate K1 for block i: add strict-lower of block i-1
            # (completing its full sum), then masked-diag of block i.
            if i > 0:
                nc.tensor.matmul(ka, maskl, kf[i - 1][:, 0:512],
                                 start=False, stop=False,
                                 skip_group_check=True)
                nc.tensor.matmul(kb, maskl, kf[i - 1][:, 512:768],
                                 start=False, stop=False,
                                 skip_group_check=True)
            nc.tensor.matmul(ka, maskd, kf[i][:, 0:512],
                             start=(i == 0), stop=(i == LT - 1),
                             skip_group_check=True)
            nc.tensor.matmul(kb, maskd, kf[i][:, 512:768],
                             start=(i == 0), stop=(i == LT - 1),
                             skip_group_check=True)

        def denstage(i):
            # den = qf . K1 per head; rq = 1/den (EPS dropped: den >= ~3e-3)
            dn = dn_p.tile([128, H * F], BF16, tag="dn", name="dn")
            nc.vector.tensor_mul(dn[:, 0:512], qf[i][:, 0:512], ka)
            nc.vector.tensor_mul(dn[:, 512:768], qf[i][:, 512:768], kb)
            den = sm_p.tile([128, 12], F32, tag="den", name="den")
            nc.vector.reduce_sum(den, dn.rearrange("p (h f) -> p h f", f=F),
                                 axis=mybir.AxisListType.X)
            rqb = sm_p.tile([128, 12], BF16, tag="rqb", name="rqb")
            with nc.allow_low_precision(reason="recip of O(100) denom"):
                nc.vector.reciprocal(rqb, den)
            nc.vector.tensor_mul(
                qf_b[i].rearrange("p (h f) -> p h f", f=F),
                qf[i].rearrange("p (h f) -> p h f", f=F),
                rqb.to_broadcast((128, 12, F)))

        def qtstage(i):
            for t in range(NH2):
                nc.sync.dma_start(
                    out=qfT[t][:, i * 128:(i + 1) * 128],
                    in_=qf_b[i][:, t * 128:(t + 1) * 128], transpose=True)

        # ---------------- attention + output projection ----------------
        def scores(i, tp, on_dve):
            """Diag-block scores for head pair-of-pairs tp (t=2tp, 2tp+1...).

            Actually per t (head pair): two 64-contraction matmuls into one
            [128, 256] psum, masked into st bf16."""
            t = tp
            pq = psts([128, 256])
            for hh in range(2):
                nc.tensor.matmul(
                    pq[:, hh * 128:(hh + 1) * 128],
                    kfT[t][hh * 64:hh * 64 + 64, i * 128:(i + 1) * 128],
                    qfT[t][hh * 64:hh * 64 + 64, i * 128:(i + 1) * 128],
                    start=True, stop=True, skip_group_check=True)
            st = st_p.tile([128, 256], BF16, tag="st", name="st")
            if on_dve:
                nc.vector.tensor_mul(st, pq, maskf)
            else:
                raw = st_p.tile([128, 256], BF16, tag="straw", name="straw")
                nc.scalar.copy(raw, pq)
                nc.gpsimd.tensor_mul(st, raw, maskf)
            return st

        def pa_pair(i, tp, sts, pas):
            """Attention for t = 2tp, 2tp+1 into one [128,256] psum."""
            pa = psts([128, 256])
            for k in range(2):
                t = 2 * tp + k
                st = sts[k]
                for hh in range(2):
                    h = 2 * t + hh
                    dst = pa[hh * 64:hh * 64 + 64, k * 128:(k + 1) * 128]
                    if i > 0:
                        nc.tensor.matmul(
                            dst,
                            NP[i - 1][hh * 64:hh * 64 + 64,
                                      t * F:(t + 1) * F],
                            qfT[t][hh * 64:hh * 64 + 64,
                                   i * 128:(i + 1) * 128],
                            start=True, stop=False, skip_group_check=True)
                    nc.tensor.matmul(
                        dst, v_p[i][:, h * F:(h + 1) * F],
                        st[:, hh * 128:(hh + 1) * 128],
                        start=(i == 0), stop=True, skip_group_check=True)
            pas.append((tp, pa))

        def aT_evac(i, tp, pa):
            nc.scalar.copy(
                aTv[:, 2 * tp:2 * tp + 2, i * 128:(i + 1) * 128],
                pa.rearrange("p (t l) -> p t l", l=128))

        def outproj(i, tp, po1, po2):
            for k in range(2):
                tt = 2 * tp + k
                st0 = zb and tt == 0
                sp = tt == NH2 - 1
                nc.tensor.matmul(po1, aT_all[tt][:, i * 128:(i + 1) * 128],
                                 wo[:, tt * 768:tt * 768 + 512],
                                 start=st0, stop=sp, skip_group_check=True)
                nc.tensor.matmul(po2, aT_all[tt][:, i * 128:(i + 1) * 128],
                                 wo[:, tt * 768 + 512:tt * 768 + 768],
                                 start=st0, stop=sp, skip_group_check=True)

        def iblk(i, extra=()):
            """Attention + output projection for block i; `extra` stages are
            interleaved to fill engine slack."""
            extra = list(extra)
            po1 = pst([128, 512])
            po2 = pst([128, 256])
            if not zb:
                nc.tensor.matmul(po1, ones1, b_orow[0:1, 0:512],
                                 start=True, stop=False,
                                 skip_group_check=True)
                nc.tensor.matmul(po2, ones1, b_orow[0:1, 512:768],
                                 start=True, stop=False,
                                 skip_group_check=True)
            pas = []
            sts01 = [scores(i, 0, True), scores(i, 1, False)]
            if extra:
                extra.pop(0)()
            pa_pair(i, 0, sts01, pas)
            aT_evac(i, 0, pas[0][1])
            outproj(i, 0, po1, po2)
            sts23 = [scores(i, 2, True), scores(i, 3, False)]
            if extra:
                extra.pop(0)()
            pa_pair(i, 1, sts23, pas)
            aT_evac(i, 1, pas[1][1])
            outproj(i, 1, po1, po2)
            sts45 = [scores(i, 4, True), scores(i, 5, False)]
            if extra:
                extra.pop(0)()
            pa_pair(i, 2, sts45, pas)
            aT_evac(i, 2, pas[2][1])
            outproj(i, 2, po1, po2)
            # output: evac + DMA (DVE/Act split to minimize the tail)
            osb = osb_p.tile([128, E], F32, tag="osb", name="osb")
            nc.vector.tensor_copy(osb[:, 0:512], po1)
            nc.scalar.copy(osb[:, 512:768], po2)
            nc.sync.dma_start(
                out=out_d[i * 128:(i + 1) * 128, 0:512],
                in_=osb[:, 0:512])
            nc.sync.dma_start(
                out=out_d[i * 128:(i + 1) * 128, 512:768],
                in_=osb[:, 512:768])
            for fn in extra:
                fn()

        def qstage(j):
            sA, sB, _ = qkv_mm(0, j)
            qstage_exp(j, sA, sB)

        def bstage(j):
            k1stage(j)
            denstage(j)
            qtstage(j)

        # ================= emission =================
        # Priority spine: k features -> q features -> K1/den/qfT chains,
        # with v projections and attention blocks filling in behind.
        mark("kstage0")
        kstage_mm(0)
        mark("qstage0")
        qstage(0)
        mark("kfac0")
        kstage_fac(0)
        mark("bstage0")
        bstage(0)
        mark("kmm1")
        kstage_mm(1)
        kstage_fac(1)
        mark("qstage1")
        qstage(1)
        mark("bstage1")
        bstage(1)
        mark("kmm23")
        kstage_mm(2)
        kstage_fac(2)
        kstage_mm(3)
        kstage_fac(3)
        mark("vstage0")
        vstage(0)
        njstage(0)
        mark("qstage2")
        qstage(2)
        mark("vstage1")
        vstage(1)
        njstage(1)
        nc.gpsimd.tensor_add(NP[1], NP[0], pnb[1])

        def ex_v2():
            vstage(2)
            njstage(2)
            nc.gpsimd.tensor_add(NP[2], NP[1], pnb[2])

        def ex_v3():
            vstage(3)

        mark("iblk0")
        iblk(0, extra=[lambda: (k1stage(2), denstage(2)),
                       lambda: qtstage(2), ex_v2])
        mark("qstage3")
        qstage(3)
        mark("iblk1")
        iblk(1, extra=[lambda: (k1stage(3), denstage(3)),
                       lambda: qtstage(3), ex_v3])
        mark("iblk2")
        iblk(2)
        mark("iblk3")
        iblk(3)

    if fix_waits:
        _fix_waits(nc)
    return nc


_CACHE = {}


def _host_consts(wsumb):
    import ml_dtypes
    bf = ml_dtypes.bfloat16
    tri = np.triu(np.ones((128, 128), dtype=np.float32))
    masks = np.concatenate(
        [tri, np.tril(np.ones((128, 128), dtype=np.float32), -1),
         np.tile(tri, (1, 2))], axis=1).astype(bf)
    return {"consts": np.concatenate([masks, wsumb], axis=1)}


def _pair_pack(w, cols):
    """[768, cols] -> [128, PR*2*cols] fp8 e-pair/plane-major layout."""
    import ml_dtypes
    f8 = ml_dtypes.float8_e4m3
    return np.ascontiguousarray(
        w.reshape(PR, 2, 128, cols).transpose(2, 0, 1, 3)
        .reshape(128, PR * 2 * cols)).astype(f8)


def _in_maps(x, w_inp, b_inp, w_out, b_out, omega):
    import ml_dtypes
    bf = ml_dtypes.bfloat16
    f = lambda a: np.ascontiguousarray(np.asarray(a), dtype=np.float32)
    x, w_inp, b_inp = f(x), f(w_inp), f(b_inp)
    w_out, b_out, omega = f(w_out), f(b_out), f(omega)
    w = w_inp[0]  # [E, 3E]
    omt = (omega.T * (float(Dh) ** -0.25)).astype(np.float64)   # [d, f]
    # fold omega into the q/k projections: Ws[:, (qk,h,f)] per head
    ws = np.empty((E, 1536), np.float64)
    wqk_full = w[:, 0:1536].astype(np.float64)
    for qk in range(2):
        for h in range(H):
            c = qk * 768 + h * 64
            ws[:, c:c + 64] = wqk_full[:, c:c + 64] @ omt
    # k-side per-head column sums (diag), padded 12->16, bf16 et-major
    wsum_full = np.zeros((E, 16), np.float64)
    wsum_full[:, 0:12] = ws[:, 768:1536].reshape(E, 12, 64).sum(axis=2)
    wqk8 = _pair_pack((ws * W8SCALE).astype(np.float32), 1536)
    wsumb = np.ascontiguousarray(
        wsum_full.astype(np.float32).reshape(ET, 128, 16)
        .transpose(1, 0, 2).reshape(128, ET * 16)).astype(bf)
    consts = _host_consts(wsumb)
    wv8 = _pair_pack(w[:, 1536:2304] * W8SCALE, 768)
    # bf16 v weights (block 0), same x64 scale so v'=64v uniformly
    wvb = np.ascontiguousarray(
        (w[:, 1536:2304] * W8SCALE).reshape(ET, 128, 768)
        .transpose(1, 0, 2).reshape(128, ET * 768)).astype(bf)
    # wo/64 un-does the v'=64v scaling
    wo = np.ascontiguousarray(
        (w_out[0] * IS).reshape(ET, 128, 768).transpose(1, 0, 2)
        .reshape(128, ET * 768)).astype(bf)
    zb = bool(np.all(b_inp == 0.0) and np.all(b_out == 0.0))
    maps = []
    for c in range(B):
        xT = x[c].T
        x8 = _pair_pack(xT, L)
        xbn = np.ascontiguousarray(
            xT.reshape(ET, 128, L).transpose(1, 0, 2)
            .reshape(128, ET * L)).astype(bf)
        m = {"x8": x8, "xb": xbn, "wqk8": wqk8, "wv8": wv8, "wvb": wvb,
             "wo": wo}
        if not zb:
            bs = np.zeros((2, 1536 + 16), np.float32)
            for qk in range(2):
                bq = b_inp[qk * 768:(qk + 1) * 768].astype(np.float64)
                bsh = np.empty((768,), np.float64)
                for h in range(H):
                    bsh[h * 64:(h + 1) * 64] = bq[h * 64:(h + 1) * 64] @ omt
                # bias rows feed the x64-scaled psum: multiply by 64;
                # the pd psum is unscaled bf16: sums stay unscaled
                bs[qk, 0:768] = (bsh * W8SCALE).astype(np.float32)
                if qk == 1:
                    bs[1, 1536:1548] = (
                        bsh.reshape(12, 64).sum(axis=1)).astype(np.float32)
            m["bs_rows"] = bs
            m["ones1"] = np.ones((1, 128), np.float32)
            m["b_vv"] = np.ascontiguousarray(np.broadcast_to(
                b_inp[1536:2304] * W8SCALE, (128, E))).astype(np.float32)
            m["b_orow"] = np.ascontiguousarray(b_out).reshape(1, E)
        m.update(consts)
        maps.append(m)
    return maps


def kernel(x, w_inp, b_inp, w_out, b_out, omega):
    maps = _in_maps(x, w_inp, b_inp, w_out, b_out, omega)
    zb = "b_vv" not in maps[0]
    key = f"nc{int(zb)}"
    if key not in _CACHE:
        _CACHE[key] = build_nc(zb=zb)
    nc = _CACHE[key]
    res = bass_utils.run_bass_kernel_spmd(nc, maps, core_ids=list(range(B)))
    return np.stack([res.results[c]["out"] for c in range(B)])


# revision 38
# speedup vs baseline: 1.2492x; 1.0854x over previous
"""Trainium2 Bass kernel v3: FAVOR (Performer) causal linear attention block.

Per batch element (data-parallel over 8 NeuronCores):
  c = x @ w_inp + b_inp; q,k,v = split(c)
  qf/kf = rfm_softmax(q/k, omega)             (FAVOR random feature maps)
  a     = causal_linear_attention(qf, kf, v)  (prefix outer-products + masked
                                               diagonal blocks)
  out   = a @ w_out + b_out

v3 design notes:
  - x transposed on host; QKV-feature and V GEMMs run as fp8e4 DoubleRow
    matmuls (2 k-planes per instruction, 0.5 cyc/row); weights pre-scaled
    by 64 into fp8 normal range, un-scaled via exp(s/64) activation scale
    (features) and wo/64 host fold (v path: v'=64v carried through).
  - q-side normalizer exp(-diag-m)/sqrt(F) cancels in a/denom: qf = exp(s_q).
  - k-side max taken as r = rowmax(exp(s_k)) on the bf16 feature tile;
    per-head factor applied as one broadcast DVE multiply.
  - K1 (cumulative kf sums) accumulated in a persistent PSUM pair via
    triu/strict-tril masks: 2 matmuls per block after the first.
  - attention: per-block diag scores (masked on DVE/Pool) + prefix NP
    outer-product matmuls; aT feeds output projection directly as lhsT.
"""

import numpy as np
from contextlib import ExitStack

import concourse.bass as bass
import concourse.tile as tile
from concourse import mybir
from concourse import bass_utils
import bass_rust

F32 = mybir.dt.float32
F32R = mybir.dt.float32r
BF16 = mybir.dt.bfloat16
F8 = mybir.dt.float8e4
AF = mybir.ActivationFunctionType
ALU = mybir.AluOpType
DR = mybir.MatmulPerfMode.DoubleRow

B, L, E, H, Dh, F = 8, 512, 768, 12, 64, 64
LT = L // 128       # 4 l-chunks
ET = E // 128       # 6 e-chunks
PR = ET // 2        # 3 e-pair chunks (DoubleRow planes)
NH2 = H // 2        # 6 head pairs
EPS = 1e-6
W8SCALE = 64.0
IS = 1.0 / W8SCALE

PHASES = []         # (name, first_instruction_number) markers for profiling


def _fix_waits(nc, cap=1):
    """Walrus codegen allows a single sync-wait per instruction; hoist excess
    waits onto injected same-engine NoOps placed directly before the offender
    (no reordering, deadlock-free)."""
    n = 0
    for fn in nc.m.functions:
        for bb in fn.blocks:
            insts = bb.instructions
            i = 0
            while i < len(insts):
                inst = insts[i]
                si = inst.sync_info
                if si is not None:
                    ow = list(si.on_wait)
                    if len(ow) > cap:
                        excess, keep = ow[:-cap], ow[-cap:]
                        si.on_wait = keep
                        for w in excess:
                            n += 1
                            nop = bass_rust.InstNoOp(
                                name=f"waitnop_{n}",
                                engine=inst.engine,
                                sync_info=bass_rust.SyncInfo(
                                    on_wait=[w], on_update=[]),
                            )
                            insts.insert(i, nop)
                            i += 1
                i += 1
    return n


def build_nc(fix_waits=True, zb=True):
    nc = bass.Bass("TRN2", target_bir_lowering=False, debug=False,
                   num_devices=8)
    PHASES.clear()

    def mark(name):
        PHASES.append((name, int(nc.get_next_instruction_name()[2:])))

    x8_d = nc.dram_tensor("x8", [128, PR * 2 * L], F8,
                          kind="ExternalInput").ap()
    xb_d = nc.dram_tensor("xb", [128, ET * L], BF16,
                          kind="ExternalInput").ap()
    wqk8_d = nc.dram_tensor("wqk8", [128, PR * 2 * 1536], F8,
                            kind="ExternalInput").ap()
    wvb_d = nc.dram_tensor("wvb", [128, ET * 768], BF16,
                           kind="ExternalInput").ap()
    wv8_d = nc.dram_tensor("wv8", [128, PR * 2 * 768], F8,
                           kind="ExternalInput").ap()
    wo_d = nc.dram_tensor("wo", [128, ET * 768], BF16,
                          kind="ExternalInput").ap()
    consts_d = nc.dram_tensor("consts", [128, 768 + ET * 16], BF16,
                              kind="ExternalInput").ap()
    if not zb:
        ones1_d = nc.dram_tensor("ones1", [1, 128], F32R,
                                 kind="ExternalInput").ap()
        bs_d = nc.dram_tensor("bs_rows", [2, 1536 + 16], F32R,
                              kind="ExternalInput").ap()
        b_vv_d = nc.dram_tensor("b_vv", [128, E], F32,
                                kind="ExternalInput").ap()
        b_orow_d = nc.dram_tensor("b_orow", [1, E], F32R,
                                  kind="ExternalInput").ap()
    out_d = nc.dram_tensor("out", [L, E], F32, kind="ExternalOutput").ap()

    with tile.TileContext(nc) as tc, ExitStack() as ctx:
        P = ctx.enter_context(tc.tile_pool(name="persist", bufs=1))
        st_p = ctx.enter_context(tc.tile_pool(name="stp", bufs=6))
        sm_p = ctx.enter_context(tc.tile_pool(name="smp", bufs=8))
        dn_p = ctx.enter_context(tc.tile_pool(name="dnp", bufs=2))
        osb_p = ctx.enter_context(tc.tile_pool(name="osb", bufs=2))
        ps = ctx.enter_context(tc.tile_pool(name="ps", bufs=1, space="PSUM"))

        cnt = [0]

        def pst(shape, dtype=F32, tag="big", bufs=4):
            cnt[0] += 1
            return ps.tile(shape, dtype, tag=tag, bufs=bufs,
                           name=f"pst{cnt[0]}")

        def psts(shape, dtype=F32):
            return pst(shape, dtype, tag="small", bufs=2)

        # PSUM budget: tag big x4 + small x2 + acc x2 = 8 banks.

        # Act-table warmup: absorb the 1.3us activation table load at t=0
        warm = P.tile([128, 1], F32, tag="warm", name="warm")
        nc.gpsimd.memset(warm, 0.0)
        nc.scalar.activation(warm, warm, AF.Exp)

        # ---------------- DMAs ----------------
        # SP queue spine, in critical-path order: x8, k-side weights, bf16 x
        # (pd), q-side weights. Strided q/k-half DMAs keep transfers minimal.
        x8 = P.tile([128, PR * 2 * L], F8, tag="x8", name="x8")
        x8v = x8.rearrange("p (pr two l) -> p pr two l", two=2, l=L)
        wqk8 = P.tile([128, PR * 2 * 1536], F8, tag="wqk8", name="wqk8")
        wqk8v = wqk8.rearrange("p (pr two c) -> p pr two c", two=2, c=1536)
        wqk8dv = wqk8_d.rearrange("p (pr two c) -> p pr two c", two=2, c=1536)
        xb = P.tile([128, ET * L], BF16, tag="xb", name="xb")
        xbv = xb.rearrange("p (et l) -> p et l", l=L)
        nc.sync.dma_start(out=x8, in_=x8_d)
        nc.sync.dma_start(out=wqk8v[:, :, :, 768:1536],
                          in_=wqk8dv[:, :, :, 768:1536])
        nc.sync.dma_start(out=xb, in_=xb_d)
        nc.sync.dma_start(out=wqk8v[:, :, :, 0:768],
                          in_=wqk8dv[:, :, :, 0:768])
        if not zb:
            ones1 = P.tile([1, 128], F32R, tag="ones1", name="ones1")
            nc.sync.dma_start(out=ones1, in_=ones1_d)
            bs_rows = P.tile([2, 1536 + 16], F32R, tag="bs_rows",
                             name="bs_rows")
            nc.sync.dma_start(out=bs_rows, in_=bs_d)

        # Pool (SWDGE) queue: few big DMAs (SWDGE prep ~1us each serializes
        # the queue) in need order: masks+wsum, wvb, wo, wv8.
        consts = P.tile([128, 768 + ET * 16], BF16, tag="consts",
                        name="consts")
        nc.gpsimd.dma_start(out=consts, in_=consts_d)
        maskd = consts[:, 0:128]
        maskl = consts[:, 128:256]
        maskf4 = consts[:, 256:768]
        wsumb = consts[:, 768:768 + ET * 16]
        wvb = P.tile([128, ET * 768], BF16, tag="wvb", name="wvb")
        nc.gpsimd.dma_start(out=wvb, in_=wvb_d)
        wo = P.tile([128, ET * 768], BF16, tag="wo", name="wo")
        nc.gpsimd.dma_start(out=wo, in_=wo_d)
        wv8 = P.tile([128, PR * 2 * 768], F8, tag="wv8", name="wv8")
        wv8v = wv8.rearrange("p (pr two c) -> p pr two c", two=2, c=768)
        nc.gpsimd.dma_start(out=wv8, in_=wv8_d)
        if not zb:
            b_vv = P.tile([128, E], F32, tag="b_vv", name="b_vv")
            nc.gpsimd.dma_start(out=b_vv, in_=b_vv_d)
            b_orow = P.tile([1, E], F32R, tag="b_orow", name="b_orow")
            nc.gpsimd.dma_start(out=b_orow, in_=b_orow_d)

        # ---------------- persistent SBUF tiles ----------------
        kf = [P.tile([128, H * F], BF16, tag=f"kf{lt}", name=f"kf{lt}")
              for lt in range(LT)]
        qf = [P.tile([128, H * F], BF16, tag=f"qf{lt}", name=f"qf{lt}")
              for lt in range(LT)]
        qf_b = [P.tile([128, H * F], BF16, tag=f"qfb{lt}", name=f"qfb{lt}")
                for lt in range(LT)]
        v_p = [P.tile([128, E], BF16, tag=f"vp{lt}", name=f"vp{lt}")
               for lt in range(LT)]
        kfT_all = P.tile([128, NH2 * L], BF16, tag="kfT", name="kfT")
        kfT = [kfT_all[:, t * L:(t + 1) * L] for t in range(NH2)]
        qfT_all = P.tile([128, NH2 * L], BF16, tag="qfT", name="qfT")
        qfT = [qfT_all[:, t * L:(t + 1) * L] for t in range(NH2)]
        aTbig = P.tile([128, NH2 * L], BF16, tag="aT", name="aT")
        aT_all = [aTbig[:, t * L:(t + 1) * L] for t in range(NH2)]
        aTv = aTbig.rearrange("p (t l) -> p t l", l=L)
        # NP prefix outer products: NPs[j] = sum_{j'<=j} kf_j'^T v'_j',
        # laid out [128 (hh*64+f), NH2*F (t,d)]
        pnb = [P.tile([128, NH2 * F], BF16, tag=f"pnb{j}", name=f"pnb{j}")
               for j in range(LT - 1)]
        NPs = [P.tile([128, NH2 * F], BF16, tag=f"NP{j}", name=f"NP{j}")
               for j in range(1, LT - 1)]
        NP = [pnb[0]] + NPs  # NP[j] = prefix through block j

        # persistent K1 accumulator (2 banks)
        ka = ps.tile([128, 512], F32, tag="acc", bufs=2, name="ka")
        kb = ps.tile([128, 256], F32, tag="acc", bufs=2, name="kb")

        # ---------------- feature stage ----------------
        def qkv_mm(qk, lt, with_pd=False):
            """s[l, cols] = x @ Ws via fp8 DoubleRow; returns (sA, sB, _)."""
            sA = pst([128, 512])
            sB = pst([128, 256])
            c0 = qk * 768
            if not zb:
                nc.tensor.matmul(sA, ones1, bs_rows[qk:qk + 1, 0:512],
                                 start=True, stop=False,
                                 skip_group_check=True)
                nc.tensor.matmul(sB, ones1, bs_rows[qk:qk + 1, 512:768],
                                 start=True, stop=False,
                                 skip_group_check=True)
            for p in range(PR):
                st0 = (p == 0) and zb
                sp = (p == PR - 1)
                lhs = x8v[:, p, :, lt * 128:(lt + 1) * 128]
                nc.tensor.matmul(sA, lhs, wqk8v[:, p, :, c0:c0 + 512],
                                 start=st0, stop=sp, perf_mode=DR,
                                 skip_group_check=True)
                nc.tensor.matmul(sB, lhs, wqk8v[:, p, :, c0 + 512:c0 + 768],
                                 start=st0, stop=sp, perf_mode=DR,
                                 skip_group_check=True)
            return sA, sB, None

        def kstage_mm(lt):
            """fp8 feature matmuls + exp; pd deferred (waits on the slower
            bf16 x load) so it doesn't block the PE queue."""
            sA, sB, _ = qkv_mm(1, lt, False)
            dst = kf[lt]
            # kf_raw = exp(s) (scale 1/64 un-does the fp8 weight scaling)
            nc.scalar.activation(dst[:, 0:512], sA, AF.Exp, scale=IS)
            nc.scalar.activation(dst[:, 512:768], sB, AF.Exp, scale=IS)

        def kstage_fac(lt):
            dst = kf[lt]
            pd = psts([128, 16])
            if not zb:
                nc.tensor.matmul(pd, ones1, bs_rows[1:2, 1536:1552],
                                 start=True, stop=False,
                                 skip_group_check=True)
            for et in range(ET):
                nc.tensor.matmul(pd, xbv[:, et, lt * 128:(lt + 1) * 128],
                                 wsumb[:, et * 16:(et + 1) * 16],
                                 start=(et == 0) and zb, stop=(et == ET - 1),
                                 skip_group_check=True)
            # r = rowmax(kf_raw) = exp(m);  fac = exp(-diag)/r
            # odd blocks run the scale multiply on Pool to unload DVE
            heavy = nc.vector if lt % 2 == 0 else nc.gpsimd
            r = sm_p.tile([128, 1], F32, tag="r", name="r")
            nc.vector.reduce_max(r, dst, axis=mybir.AxisListType.X)
            fac = sm_p.tile([128, 12], F32, tag="fac", name="fac")
            # diag = 0.5 * pd (pd unscaled bf16 path)  ->  exp(-pd/2)
            nc.scalar.activation(fac, pd[:, 0:12], AF.Exp, scale=-0.5)
            rr = sm_p.tile([128, 1], F32, tag="rr", name="rr")
            with nc.allow_low_precision(reason="recip of exp(max), O(1)"):
                nc.vector.reciprocal(rr, r)
            facb = sm_p.tile([128, 12], BF16, tag="facb", name="facb")
            nc.vector.tensor_mul(facb, fac, rr.to_broadcast((128, 12)))
            # kf = kf_raw * fac (per head broadcast)
            heavy.tensor_mul(
                dst.rearrange("p (h f) -> p h f", f=F),
                dst.rearrange("p (h f) -> p h f", f=F),
                facb.to_broadcast((128, 12, F)))
            for t in range(NH2):
                nc.sync.dma_start(
                    out=kfT[t][:, lt * 128:(lt + 1) * 128],
                    in_=dst[:, t * 128:(t + 1) * 128], transpose=True)

        def vstage(lt):
            """v' = 64*v. Block 0 runs bf16 (low-support early positions
            see v errors unaveraged); later blocks run fp8 DoubleRow."""
            pv1 = pst([128, 512])
            pv2 = pst([128, 256])
            if lt == 0:
                for et in range(ET):
                    st0 = et == 0
                    sp = et == ET - 1
                    lhs = xbv[:, et, lt * 128:(lt + 1) * 128]
                    nc.tensor.matmul(pv1, lhs,
                                     wvb[:, et * 768:et * 768 + 512],
                                     start=st0, stop=sp,
                                     skip_group_check=True)
                    nc.tensor.matmul(pv2, lhs,
                                     wvb[:, et * 768 + 512:(et + 1) * 768],
                                     start=st0, stop=sp,
                                     skip_group_check=True)
            else:
                for p in range(PR):
                    st0 = p == 0
                    sp = p == PR - 1
                    lhs = x8v[:, p, :, lt * 128:(lt + 1) * 128]
                    nc.tensor.matmul(pv1, lhs, wv8v[:, p, :, 0:512],
                                     start=st0, stop=sp, perf_mode=DR,
                                     skip_group_check=True)
                    nc.tensor.matmul(pv2, lhs, wv8v[:, p, :, 512:768],
                                     start=st0, stop=sp, perf_mode=DR,
                                     skip_group_check=True)
            # v' = 64*v kept scaled; un-scaled via wo/64 host fold
            if zb:
                nc.scalar.copy(v_p[lt][:, 0:512], pv1)
                nc.scalar.copy(v_p[lt][:, 512:768], pv2)
            else:
                # v' = psum + 64*b_v  (b_vv host-prescaled by 64)
                nc.vector.tensor_add(v_p[lt][:, 0:512], pv1, b_vv[:, 0:512])
                nc.vector.tensor_add(v_p[lt][:, 512:768], pv2,
                                     b_vv[:, 512:768])

        def njstage(lt):
            # N_lt[f, (t,d)] = kf_lt^T v'_lt per head, hh packed on partitions
            pn = pst([128, NH2 * F])
            for t in range(NH2):
                for hh in range(2):
                    h = 2 * t + hh
                    nc.tensor.matmul(
                        pn[hh * 64:hh * 64 + 64, t * F:(t + 1) * F],
                        kf[lt][:, h * F:(h + 1) * F],
                        v_p[lt][:, h * F:(h + 1) * F],
                        start=True, stop=True, skip_group_check=True)
            nc.scalar.copy(pnb[lt], pn)

        # ---------------- q stage (features + denominator) ----------------
        def qstage_mm(i):
            return qkv_mm(0, i, False)

        def qstage_exp(i, sA, sB):
            nc.scalar.activation(qf[i][:, 0:512], sA, AF.Exp, scale=IS)
            nc.scalar.activation(qf[i][:, 512:768], sB, AF.Exp, scale=IS)

        def k1stage(i):
            # ka/kb accumulate K1 for block i: add strict-lower of block i-1
            # (completing its full sum), then masked-diag of block i.
            if i > 0:
                nc.tensor.matmul(ka, maskl, kf[i - 1][:, 0:512],
                                 start=False, stop=False,
                                 skip_group_check=True)
                nc.tensor.matmul(kb, maskl, kf[i - 1][:, 512:768],
                                 start=False, stop=False,
                                 skip_group_check=True)
            nc.tensor.matmul(ka, maskd, kf[i][:, 0:512],
                             start=(i == 0), stop=(i == LT - 1),
                             skip_group_check=True)
            nc.tensor.matmul(kb, maskd, kf[i][:, 512:768],
                             start=(i == 0), stop=(i == LT - 1),
                             skip_group_check=True)

        def denstage(i):
            # den = qf . K1 per head; rq = 1/den (EPS dropped: den >= ~3e-3)
            dn = dn_p.tile([128, H * F], BF16, tag="dn", name="dn")
            nc.vector.tensor_mul(dn[:, 0:512], qf[i][:, 0:512], ka)
            nc.vector.tensor_mul(dn[:, 512:768], qf[i][:, 512:768], kb)
            den = sm_p.tile([128, 12], F32, tag="den", name="den")
            nc.vector.reduce_sum(den, dn.rearrange("p (h f) -> p h f", f=F),
                                 axis=mybir.AxisListType.X)
            rqb = sm_p.tile([128, 12], BF16, tag="rqb", name="rqb")
            with nc.allow_low_precision(reason="recip of O(100) denom"):
                nc.vector.reciprocal(rqb, den)
            nc.vector.tensor_mul(
                qf_b[i].rearrange("p (h f) -> p h f", f=F),
                qf[i].rearrange("p (h f) -> p h f", f=F),
                rqb.to_broadcast((128, 12, F)))

        def qtstage(i):
            for t in range(NH2):
                nc.sync.dma_start(
                    out=qfT[t][:, i * 128:(i + 1) * 128],
                    in_=qf_b[i][:, t * 128:(t + 1) * 128], transpose=True)

        # ---------------- attention + output projection ----------------
        def scores_quad(i, tp, on_dve):
            """Diag-block scores for t = 2tp, 2tp+1: four 64-contraction
            matmuls into one [128, 512] psum bank, masked in one op."""
            pq = psts([128, 512])
            for k in range(2):
                t = 2 * tp + k
                for hh in range(2):
                    nc.tensor.matmul(
                        pq[:, k * 256 + hh * 128:k * 256 + (hh + 1) * 128],
                        kfT[t][hh * 64:hh * 64 + 64, i * 128:(i + 1) * 128],
                        qfT[t][hh * 64:hh * 64 + 64, i * 128:(i + 1) * 128],
                        start=True, stop=True, skip_group_check=True)
            st = st_p.tile([128, 512], BF16, tag="st", name="st")
            if on_dve:
                nc.vector.tensor_mul(st, pq, maskf4)
            else:
                raw = st_p.tile([128, 512], BF16, tag="straw", name="straw")
                nc.scalar.copy(raw, pq)
                nc.gpsimd.tensor_mul(st, raw, maskf4)
            return st

        def pa_pair(i, tp, st, pas):
            """Attention for t = 2tp, 2tp+1 into one [128,256] psum."""
            pa = psts([128, 256])
            for k in range(2):
                t = 2 * tp + k
                for hh in range(2):
                    h = 2 * t + hh
                    dst = pa[hh * 64:hh * 64 + 64, k * 128:(k + 1) * 128]
                    if i > 0:
                        nc.tensor.matmul(
                            dst,
                            NP[i - 1][hh * 64:hh * 64 + 64,
                                      t * F:(t + 1) * F],
                            qfT[t][hh * 64:hh * 64 + 64,
                                   i * 128:(i + 1) * 128],
                            start=True, stop=False, skip_group_check=True)
                    nc.tensor.matmul(
                        dst, v_p[i][:, h * F:(h + 1) * F],
                        st[:, k * 256 + hh * 128:k * 256 + (hh + 1) * 128],
                        start=(i == 0), stop=True, skip_group_check=True)
            pas.append((tp, pa))

        def aT_evac(i, tp, pa):
            nc.scalar.copy(
                aTv[:, 2 * tp:2 * tp + 2, i * 128:(i + 1) * 128],
                pa.rearrange("p (t l) -> p t l", l=128))

        def outproj(i, tp, po1, po2):
            for k in range(2):
                tt = 2 * tp + k
                st0 = zb and tt == 0
                sp = tt == NH2 - 1
                nc.tensor.matmul(po1, aT_all[tt][:, i * 128:(i + 1) * 128],
                                 wo[:, tt * 768:tt * 768 + 512],
                                 start=st0, stop=sp, skip_group_check=True)
                nc.tensor.matmul(po2, aT_all[tt][:, i * 128:(i + 1) * 128],
                                 wo[:, tt * 768 + 512:tt * 768 + 768],
                                 start=st0, stop=sp, skip_group_check=True)

        def iblk(i, extra=()):
            """Attention + output projection for block i; `extra` stages are
            interleaved to fill engine slack."""
            extra = list(extra)
            po1 = pst([128, 512])
            po2 = pst([128, 256])
            if not zb:
                nc.tensor.matmul(po1, ones1, b_orow[0:1, 0:512],
                                 start=True, stop=False,
                                 skip_group_check=True)
                nc.tensor.matmul(po2, ones1, b_orow[0:1, 512:768],
                                 start=True, stop=False,
                                 skip_group_check=True)
            pas = []
            st0 = scores_quad(i, 0, True)
            if extra:
                extra.pop(0)()
            pa_pair(i, 0, st0, pas)
            aT_evac(i, 0, pas[0][1])
            outproj(i, 0, po1, po2)
            st1 = scores_quad(i, 1, False)
            if extra:
                extra.pop(0)()
            pa_pair(i, 1, st1, pas)
            aT_evac(i, 1, pas[1][1])
            outproj(i, 1, po1, po2)
            st2 = scores_quad(i, 2, True)
            if extra:
                extra.pop(0)()
            pa_pair(i, 2, st2, pas)
            aT_evac(i, 2, pas[2][1])
            outproj(i, 2, po1, po2)
            # output: evac + DMA (DVE/Act split + chunked DMAs to minimize
            # the kernel tail)
            osb = osb_p.tile([128, E], F32, tag="osb", name="osb")
            nc.vector.tensor_copy(osb[:, 0:512], po1)
            nc.scalar.copy(osb[:, 512:768], po2)
            nc.sync.dma_start(
                out=out_d[i * 128:(i + 1) * 128, 0:512],
                in_=osb[:, 0:512])
            nc.sync.dma_start(
                out=out_d[i * 128:(i + 1) * 128, 512:768],
                in_=osb[:, 512:768])
            for fn in extra:
                fn()

        def qstage(j):
            sA, sB, _ = qkv_mm(0, j)
            qstage_exp(j, sA, sB)

        def bstage(j):
            k1stage(j)
            denstage(j)
            qtstage(j)

        # ================= emission =================
        # Priority spine: k features -> q features -> K1/den/qfT chains,
        # with v projections and attention blocks filling in behind.
        mark("kstage0")
        kstage_mm(0)
        mark("qstage0")
        qstage(0)
        mark("kfac0")
        kstage_fac(0)
        mark("bstage0")
        bstage(0)
        mark("kmm1")
        kstage_mm(1)
        kstage_fac(1)
        mark("qstage1")
        qstage(1)
        mark("bstage1")
        bstage(1)
        mark("kmm23")
        kstage_mm(2)
        kstage_fac(2)
        kstage_mm(3)
        kstage_fac(3)
        mark("vstage0")
        vstage(0)
        njstage(0)
        mark("qstage2")
        qstage(2)
        mark("vstage1")
        vstage(1)
        njstage(1)
        nc.gpsimd.tensor_add(NP[1], NP[0], pnb[1])

        def ex_v2():
            vstage(2)
            njstage(2)
            nc.gpsimd.tensor_add(NP[2], NP[1], pnb[2])

        def ex_v3():
            vstage(3)

        mark("iblk0")
        iblk(0, extra=[lambda: (k1stage(2), denstage(2)),
                       lambda: qtstage(2), ex_v2])
        mark("qstage3")
        qstage(3)
        mark("iblk1")
        iblk(1, extra=[lambda: (k1stage(3), denstage(3)),
                       lambda: qtstage(3), ex_v3])
        mark("iblk2")
        iblk(2)
        mark("iblk3")
        iblk(3)

    if fix_waits:
        _fix_waits(nc)
    return nc


_CACHE = {}


def _host_consts(wsumb):
    import ml_dtypes
    bf = ml_dtypes.bfloat16
    tri = np.triu(np.ones((128, 128), dtype=np.float32))
    masks = np.concatenate(
        [tri, np.tril(np.ones((128, 128), dtype=np.float32), -1),
         np.tile(tri, (1, 4))], axis=1).astype(bf)
    return {"consts": np.concatenate([masks, wsumb], axis=1)}


def _pair_pack(w, cols):
    """[768, cols] -> [128, PR*2*cols] fp8 e-pair/plane-major layout."""
    import ml_dtypes
    f8 = ml_dtypes.float8_e4m3
    return np.ascontiguousarray(
        w.reshape(PR, 2, 128, cols).transpose(2, 0, 1, 3)
        .reshape(128, PR * 2 * cols)).astype(f8)


def _in_maps(x, w_inp, b_inp, w_out, b_out, omega):
    import ml_dtypes
    bf = ml_dtypes.bfloat16
    f = lambda a: np.ascontiguousarray(np.asarray(a), dtype=np.float32)
    x, w_inp, b_inp = f(x), f(w_inp), f(b_inp)
    w_out, b_out, omega = f(w_out), f(b_out), f(omega)
    w = w_inp[0]  # [E, 3E]
    omt = (omega.T * (float(Dh) ** -0.25)).astype(np.float64)   # [d, f]
    # fold omega into the q/k projections: Ws[:, (qk,h,f)] per head
    ws = np.empty((E, 1536), np.float64)
    wqk_full = w[:, 0:1536].astype(np.float64)
    for qk in range(2):
        for h in range(H):
            c = qk * 768 + h * 64
            ws[:, c:c + 64] = wqk_full[:, c:c + 64] @ omt
    # k-side per-head column sums (diag), padded 12->16, bf16 et-major
    wsum_full = np.zeros((E, 16), np.float64)
    wsum_full[:, 0:12] = ws[:, 768:1536].reshape(E, 12, 64).sum(axis=2)
    wqk8 = _pair_pack((ws * W8SCALE).astype(np.float32), 1536)
    wsumb = np.ascontiguousarray(
        wsum_full.astype(np.float32).reshape(ET, 128, 16)
        .transpose(1, 0, 2).reshape(128, ET * 16)).astype(bf)
    consts = _host_consts(wsumb)
    wv8 = _pair_pack(w[:, 1536:2304] * W8SCALE, 768)
    # bf16 v weights (block 0), same x64 scale so v'=64v uniformly
    wvb = np.ascontiguousarray(
        (w[:, 1536:2304] * W8SCALE).reshape(ET, 128, 768)
        .transpose(1, 0, 2).reshape(128, ET * 768)).astype(bf)
    # wo/64 un-does the v'=64v scaling
    wo = np.ascontiguousarray(
        (w_out[0] * IS).reshape(ET, 128, 768).transpose(1, 0, 2)
        .reshape(128, ET * 768)).astype(bf)
    zb = bool(np.all(b_inp == 0.0) and np.all(b_out == 0.0))
    maps = []
    for c in range(B):
        xT = x[c].T
        x8 = _pair_pack(xT, L)
        xbn = np.ascontiguousarray(
            xT.reshape(ET, 128, L).transpose(1, 0, 2)
            .reshape(128, ET * L)).astype(bf)
        m = {"x8": x8, "xb": xbn, "wqk8": wqk8, "wv8": wv8, "wvb": wvb,
             "wo": wo}
        if not zb:
            bs = np.zeros((2, 1536 + 16), np.float32)
            for qk in range(2):
                bq = b_inp[qk * 768:(qk + 1) * 768].astype(np.float64)
                bsh = np.empty((768,), np.float64)
                for h in range(H):
                    bsh[h * 64:(h + 1) * 64] = bq[h * 64:(h + 1) * 64] @ omt
                # bias rows feed the x64-scaled psum: multiply by 64;
                # the pd psum is unscaled bf16: sums stay unscaled
                bs[qk, 0:768] = (bsh * W8SCALE).astype(np.float32)
                if qk == 1:
                    bs[1, 1536:1548] = (
                        bsh.reshape(12, 64).sum(axis=1)).astype(np.float32)
            m["bs_rows"] = bs
            m["ones1"] = np.ones((1, 128), np.float32)
            m["b_vv"] = np.ascontiguousarray(np.broadcast_to(
                b_inp[1536:2304] * W8SCALE, (128, E))).astype(np.float32)
            m["b_orow"] = np.ascontiguousarray(b_out).reshape(1, E)
        m.update(consts)
        maps.append(m)
    return maps


def kernel(x, w_inp, b_inp, w_out, b_out, omega):
    maps = _in_maps(x, w_inp, b_inp, w_out, b_out, omega)
    zb = "b_vv" not in maps[0]
    key = f"nc{int(zb)}"
    if key not in _CACHE:
        _CACHE[key] = build_nc(zb=zb)
    nc = _CACHE[key]
    res = bass_utils.run_bass_kernel_spmd(nc, maps, core_ids=list(range(B)))
    return np.stack([res.results[c]["out"] for c in range(B)])
